# revision 15
# baseline (speedup 1.0000x reference)
"""Trainium2 Bass kernel for nn_NoiseGenerator.

Math (verified against the jax reference on host):
  The reference's irfft -> fftshift -> hann-window -> slice pipeline is a fixed
  linear map of the 8192 spectral magnitudes into a 1023-tap impulse:
      impulse = noise_bands @ C,
      C[k, j] = w_j * alpha_k * cos(2*pi*k*(7681 + j) / 16382)
      w_j = 0.5 - 0.5*cos(2*pi*(j+2)/1024),  alpha = [1, 2, ..., 2, 1] / 16382
  The fft_convolve + crop is then an ordinary linear convolution of noise row 0
  with each batch row's impulse:
      out[b, t] = amps_b / M * sum_i f[b, i] * n0u[t - 512 + i]
      f[b, i] = impulse[b, 1022 - i] (reversal folded into C), n0u = mean_0 + eps[0]
      M = max(mean + eps)  (global max over the full batch)

Distribution over 8 cores: data-parallel over batch (128 rows/core).
  - Each core computes a 128-column slice of W_ic = [W_coeff; b_coeff] @ C_rev
    (the only batch-independent heavy matmul) -> AllGather.
  - The global max M -> AllReduce(max) of one scalar.
  - Everything else is per-shard local: small MLP heads, the Toeplitz-matmul
    convolution against shifted windows of n0, output scaling folded into the
    filter via the rows of x.
"""

import os

import numpy as np

B = 1024
D = 64
DA = D + 2  # vel feats + K/MAX_STEPS + const-1 (folds the coeff bias)
W = 8192
NFFT = 16382
IRP = 1024  # impulse taps padded 1023 -> 1024
NCORES = 8
BSH = B // NCORES
MAX_STEPS = 2799.0
TPAD = 512  # left zero-padding of n0 (= (IR_SIZE-1)//2 - 1 + 2 offset fold)
N0PAD = 9216  # 512 + 8192 + 512 zeros
TW = 9088  # Toeplitz tile free size: 8192 + 7*128 + 512 - 512

_CACHE = {}


def _build_crev() -> np.ndarray:
    """C_rev [8192, 1024] f32: column i equals C[:, 1022-i]; column 1023 is 0."""
    k = np.arange(W, dtype=np.float64)
    alpha = np.full(W, 2.0)
    alpha[0] = 1.0
    alpha[-1] = 1.0
    alpha /= NFFT
    j = np.arange(1023, dtype=np.float64)
    wj = 0.5 - 0.5 * np.cos(2.0 * np.pi * (j + 2.0) / 1024.0)
    ang = (2.0 * np.pi / NFFT) * np.outer(k, 7681.0 + j)
    C = (alpha[:, None] * np.cos(ang)) * wj[None, :]
    crev = np.zeros((W, IRP), dtype=np.float64)
    crev[:, :1023] = C[:, ::-1]
    return np.ascontiguousarray(crev, dtype=np.float32)


def _build_program():
    import concourse.bass as bass
    import concourse.mybir as mybir
    import concourse.tile as tile
    from concourse import bacc, masks
    from contextlib import ExitStack

    f32 = mybir.dt.float32
    f32r = mybir.dt.float32r
    AFT = mybir.ActivationFunctionType
    X = mybir.AxisListType.X
    CAX = mybir.AxisListType.C
    RG = [list(range(NCORES))]

    nc = bacc.Bacc(
        "TRN2", target_bir_lowering=False, debug=False, num_devices=NCORES
    )

    vel = nc.dram_tensor("vel", [BSH, D], f32, kind="ExternalInput").ap()
    kk = nc.dram_tensor("kk", [BSH, 1], f32, kind="ExternalInput").ap()
    eps_sh = nc.dram_tensor("eps_sh", [BSH, W], f32, kind="ExternalInput").ap()
    eps0 = nc.dram_tensor("eps0", [1, W], f32, kind="ExternalInput").ap()
    vel0 = nc.dram_tensor("vel0", [1, D], f32, kind="ExternalInput").ap()
    k0 = nc.dram_tensor("k0", [1, 1], f32, kind="ExternalInput").ap()
    w_aug_t = nc.dram_tensor("w_aug_t", [W, DA], f32, kind="ExternalInput").ap()
    w_amps = nc.dram_tensor("w_amps", [DA, 1], f32, kind="ExternalInput").ap()
    w_mean = nc.dram_tensor("w_mean", [DA, 1], f32, kind="ExternalInput").ap()
    c_slice = nc.dram_tensor("c_slice", [W, BSH], f32, kind="ExternalInput").ap()
    out_noise = nc.dram_tensor("out_noise", [BSH, W], f32, kind="ExternalOutput").ap()
    mean_out = nc.dram_tensor("mean_out", [BSH, 1], f32, kind="ExternalOutput").ap()

    with tile.TileContext(nc) as tc, ExitStack() as ctx:
        const = ctx.enter_context(tc.tile_pool(name="const", bufs=1))
        work = ctx.enter_context(tc.tile_pool(name="work", bufs=2))
        pmisc = ctx.enter_context(tc.tile_pool(name="pmisc", bufs=2, space="PSUM"))
        pwic = ctx.enter_context(tc.tile_pool(name="pwic", bufs=1, space="PSUM"))
        pconv = ctx.enter_context(tc.tile_pool(name="pconv", bufs=3, space="PSUM"))
        pft = ctx.enter_context(tc.tile_pool(name="pft", bufs=2, space="PSUM"))
        dram = ctx.enter_context(tc.tile_pool(name="dram", bufs=1, space="DRAM"))

        ident = const.tile([128, 128], f32)
        masks.make_identity(nc, ident[:])

        # ---- x_aug = [vel, K/MAX_STEPS, 1] ----
        x_aug = const.tile([128, DA], f32)
        nc.sync.dma_start(x_aug[:, 0:D], vel)
        nc.sync.dma_start(x_aug[:, D : D + 1], kk)
        nc.scalar.mul(x_aug[:, D : D + 1], x_aug[:, D : D + 1], 1.0 / MAX_STEPS)
        nc.vector.memset(x_aug[:, D + 1 : DA], 1.0)

        # ---- x0 column (replicated batch row 0) ----
        x0T = const.tile([DA, 1], f32)
        nc.vector.memset(x0T[:], 1.0)
        nc.sync.dma_start(x0T[0:D, 0:1], vel0.rearrange("a b -> b a"))
        nc.sync.dma_start(x0T[D : D + 1, 0:1], k0)
        nc.scalar.mul(x0T[D : D + 1, :], x0T[D : D + 1, :], 1.0 / MAX_STEPS)

        wam = const.tile([DA, 1], f32)
        nc.sync.dma_start(wam[:], w_amps)
        wme = const.tile([DA, 1], f32)
        nc.sync.dma_start(wme[:], w_mean)

        # ---- x_aug^T (unscaled) for the heads ----
        xT_ps = pmisc.tile([DA, 128], f32, tag="misc")
        nc.tensor.transpose(xT_ps[:], x_aug[:], ident[:])
        xT = const.tile([DA, 128], f32)
        nc.vector.tensor_copy(xT[:], xT_ps[:])

        # ---- heads: amps = sigmoid(x@Wa), mean = tanh(x@Wm) (true fp32) ----
        amps_ps = pmisc.tile([128, 1], f32, tag="misc")
        nc.tensor.matmul(amps_ps[:], lhsT=xT[:], rhs=wam[:], start=True, stop=True)
        amps_sb = const.tile([128, 1], f32)
        nc.scalar.activation(amps_sb[:], amps_ps[:], AFT.Sigmoid)

        mean_ps = pmisc.tile([128, 1], f32, tag="misc")
        nc.tensor.matmul(mean_ps[:], lhsT=xT[:], rhs=wme[:], start=True, stop=True)
        mean_sb = const.tile([128, 1], f32)
        nc.scalar.activation(mean_sb[:], mean_ps[:], AFT.Tanh)
        nc.sync.dma_start(mean_out, mean_sb[:])

        m0_ps = pmisc.tile([1, 1], f32, tag="misc")
        nc.tensor.matmul(m0_ps[:], lhsT=x0T[:], rhs=wme[:], start=True, stop=True)
        m0_sb = const.tile([1, 1], f32)
        nc.scalar.activation(m0_sb[:], m0_ps[:], AFT.Tanh)

        # ---- local max of (mean_b + eps_b[w]) over this shard ----
        rm4 = const.tile([128, 4], f32)
        for i in range(4):
            ch = work.tile([128, 2048], f32, tag="epschunk")
            nc.sync.dma_start(ch[:], eps_sh[:, 2048 * i : 2048 * (i + 1)])
            nc.vector.reduce_max(rm4[:, i : i + 1], ch[:], axis=X)
        rm1 = const.tile([128, 1], f32)
        nc.vector.reduce_max(rm1[:], rm4[:], axis=X)
        nc.vector.tensor_add(rm1[:], rm1[:], mean_sb[:])
        rmT_ps = pmisc.tile([1, 128], f32, tag="misc")
        nc.tensor.transpose(rmT_ps[:], rm1[:], ident[:])
        lmax_sb = const.tile([1, 1], f32)
        nc.vector.reduce_max(lmax_sb[:], rmT_ps[:], axis=X)
        lmax = lmax_sb[0:1, 0:1]

        # ---- AllReduce(max) -> global M; s_b = amps_b / M ----
        cc_in_m = dram.tile([1, 1], f32)
        cc_out_m = dram.tile([1, 1], f32)
        nc.sync.dma_start(cc_in_m[:], lmax)
        nc.gpsimd.collective_compute(
            "AllReduce",
            mybir.AluOpType.max,
            replica_groups=RG,
            ins=[cc_in_m[:].opt()],
            outs=[cc_out_m[:].opt()],
        )
        gmax = const.tile([1, 1], f32)
        nc.sync.dma_start(gmax[:], cc_out_m[:])
        ones_row = const.tile([1, 128], f32)
        nc.vector.memset(ones_row[:], 1.0)
        gmax_ps = pmisc.tile([128, 1], f32, tag="misc")
        nc.tensor.matmul(gmax_ps[:], lhsT=ones_row[:], rhs=gmax[:], start=True, stop=True)
        minv = const.tile([128, 1], f32)
        nc.vector.reciprocal(minv[:], gmax_ps[:])
        s_sb = const.tile([128, 1], f32)
        nc.vector.tensor_mul(s_sb[:], amps_sb[:], minv[:])

        # ---- x scaled by s (folds amps/M into the conv filter), transposed ----
        xs = const.tile([128, DA], f32)
        nc.vector.tensor_scalar_mul(xs[:], x_aug[:], s_sb[:])
        xsT_ps = pmisc.tile([DA, 128], f32, tag="misc")
        nc.tensor.transpose(xsT_ps[:], xs[:], ident[:])
        xsT = const.tile([DA, 128], f32r)
        nc.vector.tensor_copy(xsT[:], xsT_ps[:])

        # ---- W_ic slice: [W_coeff; b_coeff] @ C_rev[:, my 128 cols] ----
        wT_all = const.tile([128, 64 * DA], f32r)
        nc.sync.dma_start(
            wT_all[:].rearrange("p (c d) -> p c d", c=64),
            w_aug_t.rearrange("(c p) d -> p c d", p=128).bitcast(f32r),
        )
        csb = const.tile([128, 64 * BSH], f32r)
        nc.sync.dma_start(
            csb[:].rearrange("p (c i) -> p c i", c=64),
            c_slice.rearrange("(c p) i -> p c i", p=128).bitcast(f32r),
        )
        wic_ps = pwic.tile([DA, 128], f32, tag="wic")
        for c in range(64):
            nc.tensor.matmul(
                wic_ps[:],
                lhsT=wT_all[:, DA * c : DA * (c + 1)],
                rhs=csb[:, BSH * c : BSH * (c + 1)],
                start=(c == 0),
                stop=(c == 63),
            )
        wic_sb = work.tile([DA, 128], f32)
        nc.vector.tensor_copy(wic_sb[:], wic_ps[:])

        # ---- AllGather W_ic slices -> full [DA, 1024] ----
        cc_in_w = dram.tile([DA, 128], f32)
        cc_out_w = dram.tile([NCORES, DA, 128], f32)
        nc.sync.dma_start(cc_in_w[:], wic_sb[:])
        nc.gpsimd.collective_compute(
            "AllGather",
            mybir.AluOpType.bypass,
            replica_groups=RG,
            ins=[cc_in_w[:].opt()],
            outs=[cc_out_w[:].opt()],
        )
        wic_full = const.tile([DA, IRP], f32r)
        nc.sync.dma_start(
            wic_full[:].rearrange("d (r i) -> d r i", r=NCORES),
            cc_out_w[:].transpose([1, 0, 2]).bitcast(f32r),
        )

        # ---- filter rows fT[i, b] = sum_d W_ic[d, i] * xs[b, d] ----
        fT = const.tile([128, IRP], f32r)
        for c in range(8):
            fp = pft.tile([128, 128], f32, tag="fp")
            nc.tensor.matmul(
                fp[:],
                lhsT=wic_full[:, 128 * c : 128 * (c + 1)],
                rhs=xsT[:],
                start=True,
                stop=True,
            )
            nc.vector.tensor_copy(fT[:, 128 * c : 128 * (c + 1)], fp[:])

        # ---- n0 (unscaled noise row 0), zero-padded, to DRAM ----
        n0p = const.tile([1, N0PAD], f32)
        nc.vector.memset(n0p[:, 0:TPAD], 0.0)
        nc.vector.memset(n0p[:, TPAD + W : N0PAD], 0.0)
        nc.sync.dma_start(n0p[:, TPAD : TPAD + W], eps0)
        nc.vector.tensor_scalar_add(
            n0p[:, TPAD : TPAD + W], n0p[:, TPAD : TPAD + W], m0_sb[:]
        )
        n0d = dram.tile([1, N0PAD], f32)
        nc.sync.dma_start(n0d[:], n0p[:])

        # ---- Toeplitz tile T[p, f] = n0pad[f + p] via overlapping DMA ----
        t_sb = const.tile([128, TW], f32r)
        toe_src = bass.AP(n0d[:].tensor, 0, [[1, 128], [1, TW]]).bitcast(f32r)
        nc.sync.dma_start(t_sb[:], toe_src)

        # ---- conv: out[b, t] = sum_i fT[i, b] * T[i%128, t + 128*(i//128)] ----
        for t in range(16):
            po = pconv.tile([128, 512], f32, tag="conv")
            for c in range(8):
                nc.tensor.matmul(
                    po[:],
                    lhsT=fT[:, 128 * c : 128 * (c + 1)],
                    rhs=t_sb[:, 128 * c + 512 * t : 128 * c + 512 * t + 512],
                    start=(c == 0),
                    stop=(c == 7),
                )
            ob = work.tile([128, 512], f32, tag="outbounce")
            nc.vector.tensor_copy(ob[:], po[:])
            nc.sync.dma_start(out_noise[:, 512 * t : 512 * (t + 1)], ob[:])

    nc.compile()
    return nc


def _get_program():
    if "nc" not in _CACHE:
        _CACHE["nc"] = _build_program()
    return _CACHE["nc"]


def _get_crev():
    if "crev" not in _CACHE:
        _CACHE["crev"] = _build_crev()
    return _CACHE["crev"]


def make_in_maps(inputs: dict) -> list:
    vel = np.ascontiguousarray(np.asarray(inputs["vel_inputs"]), dtype=np.float32)
    K = np.ascontiguousarray(np.asarray(inputs["K"]), dtype=np.float32)
    eps = np.ascontiguousarray(np.asarray(inputs["eps"]), dtype=np.float32)
    w_coeff = np.asarray(inputs["W_coeff"], dtype=np.float32)
    b_coeff = np.asarray(inputs["b_coeff"], dtype=np.float32)
    w_amps = np.asarray(inputs["W_amps"], dtype=np.float32)
    b_amps = np.asarray(inputs["b_amps"], dtype=np.float32)
    w_mean = np.asarray(inputs["W_mean"], dtype=np.float32)
    b_mean = np.asarray(inputs["b_mean"], dtype=np.float32)

    w_aug_t = np.ascontiguousarray(
        np.concatenate([w_coeff.T, b_coeff[:, None]], axis=1)
    )  # [W, DA]
    wa = np.ascontiguousarray(np.concatenate([w_amps, b_amps[:, None]], axis=0))
    wm = np.ascontiguousarray(np.concatenate([w_mean, b_mean[:, None]], axis=0))
    crev = _get_crev()

    in_maps = []
    for c in range(NCORES):
        sl = slice(BSH * c, BSH * (c + 1))
        in_maps.append(
            {
                "vel": np.ascontiguousarray(vel[sl]),
                "kk": np.ascontiguousarray(K[sl]),
                "eps_sh": np.ascontiguousarray(eps[sl]),
                "eps0": np.ascontiguousarray(eps[0:1]),
                "vel0": np.ascontiguousarray(vel[0:1]),
                "k0": np.ascontiguousarray(K[0:1]),
                "w_aug_t": w_aug_t,
                "w_amps": wa,
                "w_mean": wm,
                "c_slice": np.ascontiguousarray(crev[:, BSH * c : BSH * (c + 1)]),
            }
        )
    return in_maps


def kernel(**inputs):
    from concourse.bass_utils import run_bass_kernel_spmd

    nc = _get_program()
    in_maps = make_in_maps(inputs)
    trace = os.environ.get("NOISE_KERNEL_TRACE", "0") == "1"
    res = run_bass_kernel_spmd(
        nc, in_maps, core_ids=list(range(NCORES)), trace=trace
    )
    _CACHE["last_result"] = res
    out = np.concatenate([r["out_noise"] for r in res.results], axis=0)
    mean = np.concatenate([r["mean_out"] for r in res.results], axis=0)
    return out, mean


# revision 16
# speedup vs baseline: 1.3134x; 1.3134x over previous
"""Trainium2 Bass kernel for nn_NoiseGenerator.

Math (verified against the jax reference on host):
  The reference's irfft -> fftshift -> hann-window -> slice pipeline is a fixed
  linear map of the 8192 spectral magnitudes into a 1023-tap impulse:
      impulse = noise_bands @ C,
      C[k, j] = w_j * alpha_k * cos(2*pi*k*(7681 + j) / 16382)
      w_j = 0.5 - 0.5*cos(2*pi*(j+2)/1024),  alpha = [1, 2, ..., 2, 1] / 16382
  The fft_convolve + crop is then an ordinary linear convolution of noise row 0
  with each batch row's impulse:
      out[b, t] = amps_b / M * sum_i f[b, i] * n0u[t - 512 + i]
      f[b, i] = impulse[b, 1022 - i] (reversal folded into C), n0u = mean_0 + eps[0]
      M = max(mean + eps)  (global max over the full batch)

Distribution over 8 cores: data-parallel over batch (128 rows/core), two
phases with host-mediated exchange (no device collectives -- measured ~100us
of barrier/CC overhead for tiny payloads on this fabric):
  phase 1 (per core): MLP mean head, shard-local max of mean+eps, and a
    128-column slice of W_ic = [W_coeff; b_coeff] @ C_rev (C column-sharded).
  host: concatenates the 8 W_ic slices (pure gather), takes max of the 8
    shard maxima (8 floats), picks mean[0] from core 0's output.
  phase 2 (per core): amps head, filter rows fT = W_ic^T x_scaled^T with
    amps/M folded in, Toeplitz-matmul convolution against shifted windows of
    noise row 0, streamed to the output.
"""

import os

import numpy as np

B = 1024
D = 64
DA = D + 2  # vel feats + K/MAX_STEPS + const-1 (folds the coeff bias)
W = 8192
NFFT = 16382
IRP = 1024  # impulse taps padded 1023 -> 1024
NCORES = 8
BSH = B // NCORES
MAX_STEPS = 2799.0
TPAD = 512  # left zero-padding of n0
N0PAD = 9216  # 512 + 8192 + 512 zeros
TW = 9088  # Toeplitz tile free size: 8192 + 7*128 + 512 - 512

_CACHE = {}


def _build_crev() -> np.ndarray:
    """C_rev [8192, 1024] f32: column i equals C[:, 1022-i]; column 1023 is 0."""
    k = np.arange(W, dtype=np.float64)
    alpha = np.full(W, 2.0)
    alpha[0] = 1.0
    alpha[-1] = 1.0
    alpha /= NFFT
    j = np.arange(1023, dtype=np.float64)
    wj = 0.5 - 0.5 * np.cos(2.0 * np.pi * (j + 2.0) / 1024.0)
    ang = (2.0 * np.pi / NFFT) * np.outer(k, 7681.0 + j)
    C = (alpha[:, None] * np.cos(ang)) * wj[None, :]
    crev = np.zeros((W, IRP), dtype=np.float64)
    crev[:, :1023] = C[:, ::-1]
    return np.ascontiguousarray(crev, dtype=np.float32)


def _begin_program():
    import concourse.mybir as mybir
    import concourse.tile as tile
    from concourse import bacc

    nc = bacc.Bacc("TRN2", target_bir_lowering=False, debug=False, num_devices=NCORES)
    return nc, tile, mybir


def _build_x_aug(nc, tc, const, vel, kk, f32):
    """x_aug [128, DA] = [vel, K/MAX_STEPS, 1]."""
    x_aug = const.tile([128, DA], f32)
    nc.sync.dma_start(x_aug[:, 0:D], vel)
    nc.sync.dma_start(x_aug[:, D : D + 1], kk)
    nc.scalar.mul(x_aug[:, D : D + 1], x_aug[:, D : D + 1], 1.0 / MAX_STEPS)
    nc.vector.memset(x_aug[:, D + 1 : DA], 1.0)
    return x_aug


def _build_prog1():
    """Per-core: mean head, local max of mean+eps, W_ic column slice."""
    from contextlib import ExitStack

    nc, tile, mybir = _begin_program()
    f32 = mybir.dt.float32
    f32r = mybir.dt.float32r
    AFT = mybir.ActivationFunctionType
    X = mybir.AxisListType.X

    vel = nc.dram_tensor("vel", [BSH, D], f32, kind="ExternalInput").ap()
    kk = nc.dram_tensor("kk", [BSH, 1], f32, kind="ExternalInput").ap()
    eps_sh = nc.dram_tensor("eps_sh", [BSH, W], f32, kind="ExternalInput").ap()
    w_aug_t = nc.dram_tensor("w_aug_t", [W, DA], f32, kind="ExternalInput").ap()
    w_mean = nc.dram_tensor("w_mean", [DA, 1], f32, kind="ExternalInput").ap()
    c_slice = nc.dram_tensor("c_slice", [W, BSH], f32, kind="ExternalInput").ap()
    mean_out = nc.dram_tensor("mean_out", [BSH, 1], f32, kind="ExternalOutput").ap()
    lmax_out = nc.dram_tensor("lmax_out", [1, 1], f32, kind="ExternalOutput").ap()
    wic_out = nc.dram_tensor("wic_out", [DA, BSH], f32, kind="ExternalOutput").ap()

    from concourse import masks

    with tile.TileContext(nc) as tc, ExitStack() as ctx:
        const = ctx.enter_context(tc.tile_pool(name="const", bufs=1))
        work = ctx.enter_context(tc.tile_pool(name="work", bufs=2))
        pmisc = ctx.enter_context(tc.tile_pool(name="pmisc", bufs=2, space="PSUM"))
        pwic = ctx.enter_context(tc.tile_pool(name="pwic", bufs=1, space="PSUM"))

        # big streaming loads first so the DMA queues fill early
        wT_all = const.tile([128, 64 * DA], f32r)
        nc.sync.dma_start(
            wT_all[:].rearrange("p (c d) -> p c d", c=64),
            w_aug_t.rearrange("(c p) d -> p c d", p=128).bitcast(f32r),
        )
        csb = const.tile([128, 64 * BSH], f32r)
        nc.sync.dma_start(
            csb[:].rearrange("p (c i) -> p c i", c=64),
            c_slice.rearrange("(c p) i -> p c i", p=128).bitcast(f32r),
        )

        ident = const.tile([128, 128], f32)
        masks.make_identity(nc, ident[:])

        x_aug = _build_x_aug(nc, tc, const, vel, kk, f32)

        wme = const.tile([DA, 1], f32)
        nc.sync.dma_start(wme[:], w_mean)

        xT_ps = pmisc.tile([DA, 128], f32, tag="misc")
        nc.tensor.transpose(xT_ps[:], x_aug[:], ident[:])
        xT = const.tile([DA, 128], f32)
        nc.vector.tensor_copy(xT[:], xT_ps[:])

        mean_ps = pmisc.tile([128, 1], f32, tag="misc")
        nc.tensor.matmul(mean_ps[:], lhsT=xT[:], rhs=wme[:], start=True, stop=True)
        mean_sb = const.tile([128, 1], f32)
        nc.scalar.activation(mean_sb[:], mean_ps[:], AFT.Tanh)
        nc.sync.dma_start(mean_out, mean_sb[:])

        # local max of (mean_b + eps_b[w]) over this shard
        rm4 = const.tile([128, 4], f32)
        for i in range(4):
            ch = work.tile([128, 2048], f32, tag="epschunk")
            nc.sync.dma_start(ch[:], eps_sh[:, 2048 * i : 2048 * (i + 1)])
            nc.vector.reduce_max(rm4[:, i : i + 1], ch[:], axis=X)
        rm1 = const.tile([128, 1], f32)
        nc.vector.reduce_max(rm1[:], rm4[:], axis=X)
        nc.vector.tensor_add(rm1[:], rm1[:], mean_sb[:])
        rmT_ps = pmisc.tile([1, 128], f32, tag="misc")
        nc.tensor.transpose(rmT_ps[:], rm1[:], ident[:])
        lmax_sb = const.tile([1, 1], f32)
        nc.vector.reduce_max(lmax_sb[:], rmT_ps[:], axis=X)
        nc.sync.dma_start(lmax_out, lmax_sb[0:1, 0:1])

        # W_ic slice: [W_coeff; b_coeff] @ C_rev[:, my 128 cols]
        wic_ps = pwic.tile([DA, 128], f32, tag="wic")
        for c in range(64):
            nc.tensor.matmul(
                wic_ps[:],
                lhsT=wT_all[:, DA * c : DA * (c + 1)],
                rhs=csb[:, BSH * c : BSH * (c + 1)],
                start=(c == 0),
                stop=(c == 63),
            )
        wic_sb = work.tile([DA, 128], f32)
        nc.vector.tensor_copy(wic_sb[:], wic_ps[:])
        nc.sync.dma_start(wic_out, wic_sb[:])

    nc.compile()
    return nc


def _build_prog2():
    """Per-core: amps head, filter rows, Toeplitz-matmul convolution."""
    from contextlib import ExitStack

    import concourse.bass as bass

    nc, tile, mybir = _begin_program()
    f32 = mybir.dt.float32
    f32r = mybir.dt.float32r
    AFT = mybir.ActivationFunctionType

    vel = nc.dram_tensor("vel", [BSH, D], f32, kind="ExternalInput").ap()
    kk = nc.dram_tensor("kk", [BSH, 1], f32, kind="ExternalInput").ap()
    vel0 = nc.dram_tensor("vel0", [1, D], f32, kind="ExternalInput").ap()
    k0 = nc.dram_tensor("k0", [1, 1], f32, kind="ExternalInput").ap()
    eps0 = nc.dram_tensor("eps0", [1, W], f32, kind="ExternalInput").ap()
    w_amps = nc.dram_tensor("w_amps", [DA, 1], f32, kind="ExternalInput").ap()
    w_mean = nc.dram_tensor("w_mean", [DA, 1], f32, kind="ExternalInput").ap()
    m_in = nc.dram_tensor("m_in", [1, 1], f32, kind="ExternalInput").ap()
    wic_full = nc.dram_tensor("wic_full", [DA, IRP], f32, kind="ExternalInput").ap()
    out_noise = nc.dram_tensor("out_noise", [BSH, W], f32, kind="ExternalOutput").ap()

    from concourse import masks

    with tile.TileContext(nc) as tc, ExitStack() as ctx:
        const = ctx.enter_context(tc.tile_pool(name="const", bufs=1))
        work = ctx.enter_context(tc.tile_pool(name="work", bufs=3))
        pmisc = ctx.enter_context(tc.tile_pool(name="pmisc", bufs=2, space="PSUM"))
        pconv = ctx.enter_context(tc.tile_pool(name="pconv", bufs=4, space="PSUM"))
        pft = ctx.enter_context(tc.tile_pool(name="pft", bufs=2, space="PSUM"))
        dram = ctx.enter_context(tc.tile_pool(name="dram", bufs=1, space="DRAM"))

        ident = const.tile([128, 128], f32)
        masks.make_identity(nc, ident[:])

        x_aug = _build_x_aug(nc, tc, const, vel, kk, f32)

        # x0 column (replicated batch row 0) for mean_0
        x0T = const.tile([DA, 1], f32)
        nc.vector.memset(x0T[:], 1.0)
        nc.sync.dma_start(x0T[0:D, 0:1], vel0.rearrange("a b -> b a"))
        nc.sync.dma_start(x0T[D : D + 1, 0:1], k0)
        nc.scalar.mul(x0T[D : D + 1, :], x0T[D : D + 1, :], 1.0 / MAX_STEPS)

        wam = const.tile([DA, 1], f32)
        nc.sync.dma_start(wam[:], w_amps)
        wme = const.tile([DA, 1], f32)
        nc.sync.dma_start(wme[:], w_mean)

        xT_ps = pmisc.tile([DA, 128], f32, tag="misc")
        nc.tensor.transpose(xT_ps[:], x_aug[:], ident[:])
        xT = const.tile([DA, 128], f32)
        nc.vector.tensor_copy(xT[:], xT_ps[:])

        amps_ps = pmisc.tile([128, 1], f32, tag="misc")
        nc.tensor.matmul(amps_ps[:], lhsT=xT[:], rhs=wam[:], start=True, stop=True)
        amps_sb = const.tile([128, 1], f32)
        nc.scalar.activation(amps_sb[:], amps_ps[:], AFT.Sigmoid)

        m0_ps = pmisc.tile([1, 1], f32, tag="misc")
        nc.tensor.matmul(m0_ps[:], lhsT=x0T[:], rhs=wme[:], start=True, stop=True)
        m0_sb = const.tile([1, 1], f32)
        nc.scalar.activation(m0_sb[:], m0_ps[:], AFT.Tanh)

        # s_b = amps_b / M  (M broadcast to all partitions via ones-matmul)
        gmax = const.tile([1, 1], f32)
        nc.sync.dma_start(gmax[:], m_in)
        ones_row = const.tile([1, 128], f32)
        nc.vector.memset(ones_row[:], 1.0)
        gmax_ps = pmisc.tile([128, 1], f32, tag="misc")
        nc.tensor.matmul(gmax_ps[:], lhsT=ones_row[:], rhs=gmax[:], start=True, stop=True)
        minv = const.tile([128, 1], f32)
        nc.vector.reciprocal(minv[:], gmax_ps[:])
        s_sb = const.tile([128, 1], f32)
        nc.vector.tensor_mul(s_sb[:], amps_sb[:], minv[:])

        # x scaled by s (folds amps/M into the conv filter), transposed
        xs = const.tile([128, DA], f32)
        nc.vector.tensor_scalar_mul(xs[:], x_aug[:], s_sb[:])
        xsT_ps = pmisc.tile([DA, 128], f32, tag="misc")
        nc.tensor.transpose(xsT_ps[:], xs[:], ident[:])
        xsT = const.tile([DA, 128], f32r)
        nc.vector.tensor_copy(xsT[:], xsT_ps[:])

        wic_sb = const.tile([DA, IRP], f32r)
        nc.sync.dma_start(wic_sb[:], wic_full.bitcast(f32r))

        # filter rows fT[i, b] = sum_d W_ic[d, i] * xs[b, d]
        fT = const.tile([128, IRP], f32r)
        for c in range(8):
            fp = pft.tile([128, 128], f32, tag="fp")
            nc.tensor.matmul(
                fp[:],
                lhsT=wic_sb[:, 128 * c : 128 * (c + 1)],
                rhs=xsT[:],
                start=True,
                stop=True,
            )
            nc.vector.tensor_copy(fT[:, 128 * c : 128 * (c + 1)], fp[:])

        # n0 (unscaled noise row 0), zero-padded, to DRAM
        n0p = const.tile([1, N0PAD], f32)
        nc.vector.memset(n0p[:, 0:TPAD], 0.0)
        nc.vector.memset(n0p[:, TPAD + W : N0PAD], 0.0)
        nc.sync.dma_start(n0p[:, TPAD : TPAD + W], eps0)
        nc.vector.tensor_scalar_add(
            n0p[:, TPAD : TPAD + W], n0p[:, TPAD : TPAD + W], m0_sb[:]
        )
        n0d = dram.tile([1, N0PAD], f32)
        nc.sync.dma_start(n0d[:], n0p[:])

        # Toeplitz tile T[p, f] = n0pad[f + p] via overlapping DMA
        t_sb = const.tile([128, TW], f32r)
        toe_src = bass.AP(n0d[:].tensor, 0, [[1, 128], [1, TW]]).bitcast(f32r)
        nc.sync.dma_start(t_sb[:], toe_src)

        # conv: out[b, t] = sum_i fT[i, b] * T[i%128, t + 128*(i//128)]
        for t in range(16):
            po = pconv.tile([128, 512], f32, tag="conv")
            for c in range(8):
                nc.tensor.matmul(
                    po[:],
                    lhsT=fT[:, 128 * c : 128 * (c + 1)],
                    rhs=t_sb[:, 128 * c + 512 * t : 128 * c + 512 * t + 512],
                    start=(c == 0),
                    stop=(c == 7),
                )
            ob = work.tile([128, 512], f32, tag="outbounce")
            nc.vector.tensor_copy(ob[:], po[:])
            nc.sync.dma_start(out_noise[:, 512 * t : 512 * (t + 1)], ob[:])

    nc.compile()
    return nc


def _get_progs():
    if "nc1" not in _CACHE:
        _CACHE["nc1"] = _build_prog1()
        _CACHE["nc2"] = _build_prog2()
    return _CACHE["nc1"], _CACHE["nc2"]


def _get_crev():
    if "crev" not in _CACHE:
        _CACHE["crev"] = _build_crev()
    return _CACHE["crev"]


def _prep(inputs: dict) -> dict:
    p = {}
    p["vel"] = np.ascontiguousarray(np.asarray(inputs["vel_inputs"]), dtype=np.float32)
    p["K"] = np.ascontiguousarray(np.asarray(inputs["K"]), dtype=np.float32)
    p["eps"] = np.ascontiguousarray(np.asarray(inputs["eps"]), dtype=np.float32)
    w_coeff = np.asarray(inputs["W_coeff"], dtype=np.float32)
    b_coeff = np.asarray(inputs["b_coeff"], dtype=np.float32)
    p["w_aug_t"] = np.ascontiguousarray(
        np.concatenate([w_coeff.T, b_coeff[:, None]], axis=1)
    )
    p["w_amps"] = np.ascontiguousarray(
        np.concatenate(
            [np.asarray(inputs["W_amps"], np.float32), np.asarray(inputs["b_amps"], np.float32)[:, None]],
            axis=0,
        )
    )
    p["w_mean"] = np.ascontiguousarray(
        np.concatenate(
            [np.asarray(inputs["W_mean"], np.float32), np.asarray(inputs["b_mean"], np.float32)[:, None]],
            axis=0,
        )
    )
    return p


def make_in_maps1(p: dict) -> list:
    crev = _get_crev()
    maps = []
    for c in range(NCORES):
        sl = slice(BSH * c, BSH * (c + 1))
        maps.append(
            {
                "vel": np.ascontiguousarray(p["vel"][sl]),
                "kk": np.ascontiguousarray(p["K"][sl]),
                "eps_sh": np.ascontiguousarray(p["eps"][sl]),
                "w_aug_t": p["w_aug_t"],
                "w_mean": p["w_mean"],
                "c_slice": np.ascontiguousarray(crev[:, BSH * c : BSH * (c + 1)]),
            }
        )
    return maps


def glue12(results1: list) -> tuple:
    """Host-side exchange: gather W_ic slices, max of shard maxima, mean row 0."""
    wic_full = np.ascontiguousarray(
        np.concatenate([r["wic_out"] for r in results1], axis=1)
    )  # [DA, 1024]
    m = np.max([r["lmax_out"][0, 0] for r in results1]).reshape(1, 1).astype(np.float32)
    mean = np.concatenate([r["mean_out"] for r in results1], axis=0)  # [B, 1]
    return wic_full, m, mean


def make_in_maps2(p: dict, wic_full: np.ndarray, m: np.ndarray) -> list:
    maps = []
    for c in range(NCORES):
        sl = slice(BSH * c, BSH * (c + 1))
        maps.append(
            {
                "vel": np.ascontiguousarray(p["vel"][sl]),
                "kk": np.ascontiguousarray(p["K"][sl]),
                "vel0": np.ascontiguousarray(p["vel"][0:1]),
                "k0": np.ascontiguousarray(p["K"][0:1]),
                "eps0": np.ascontiguousarray(p["eps"][0:1]),
                "w_amps": p["w_amps"],
                "w_mean": p["w_mean"],
                "m_in": m,
                "wic_full": wic_full,
            }
        )
    return maps


def kernel(**inputs):
    from concourse.bass_utils import run_bass_kernel_spmd

    nc1, nc2 = _get_progs()
    p = _prep(inputs)
    trace = os.environ.get("NOISE_KERNEL_TRACE", "0") == "1"
    core_ids = list(range(NCORES))

    res1 = run_bass_kernel_spmd(nc1, make_in_maps1(p), core_ids=core_ids, trace=trace)
    wic_full, m, mean = glue12(res1.results)
    res2 = run_bass_kernel_spmd(
        nc2, make_in_maps2(p, wic_full, m), core_ids=core_ids, trace=trace
    )
    _CACHE["last_result1"] = res1
    _CACHE["last_result2"] = res2
    out = np.concatenate([r["out_noise"] for r in res2.results], axis=0)
    return out, mean


# revision 17
# speedup vs baseline: 1.6668x; 1.2690x over previous
"""Trainium2 Bass kernel for nn_NoiseGenerator.

Math (verified against the jax reference on host):
  The reference's irfft -> fftshift -> hann-window -> slice pipeline is a fixed
  linear map of the 8192 spectral magnitudes into a 1023-tap impulse:
      impulse = noise_bands @ C,
      C[k, j] = w_j * alpha_k * cos(2*pi*k*(7681 + j) / 16382)
      w_j = 0.5 - 0.5*cos(2*pi*(j+2)/1024),  alpha = [1, 2, ..., 2, 1] / 16382
  The fft_convolve + crop is then an ordinary linear convolution of noise row 0
  with each batch row's impulse:
      out[b, t] = amps_b / M * sum_i f[b, i] * n0u[t - 512 + i]
      f[b, i] = impulse[b, 1022 - i] (reversal folded into C), n0u = mean_0 + eps[0]
      M = max(mean + eps)  (global max over the full batch)

Distribution over 8 cores: data-parallel over batch (128 rows/core), two
phases with host-mediated exchange (no device collectives -- measured ~100us
of barrier/CC overhead for tiny payloads on this fabric):
  phase 1 (per core): MLP mean head, shard-local max of mean+eps, and a
    128-column slice of W_ic = [W_coeff; b_coeff] @ C_rev (C column-sharded,
    fp16 with a 2^13 scale, unscaled on-device after the accumulation).
  host: concatenates the 8 W_ic slices (pure gather), takes max of the 8
    shard maxima (8 floats), picks mean[0] from core 0's output.
  phase 2 (per core): amps head, filter rows fT = W_ic^T x_scaled^T with
    amps/M (and a 2^8 fp16-headroom scale) folded in, Toeplitz-matmul
    convolution against fp16 shifted windows of noise row 0, 2^-8 applied on
    the PSUM drain, streamed to the output.
"""

import os

import numpy as np

B = 1024
D = 64
DA = D + 2  # vel feats + K/MAX_STEPS + const-1 (folds the coeff bias)
W = 8192
NFFT = 16382
IRP = 1024  # impulse taps padded 1023 -> 1024
NCORES = 8
BSH = B // NCORES
MAX_STEPS = 2799.0
TPAD = 512  # left zero-padding of n0
N0PAD = 9216  # 512 + 8192 + 512 zeros
TW = 9088  # Toeplitz tile free size: 8192 + 7*128 + 512 - 512
CSCALE = 8192.0  # fp16 range lift for C (|C| <= 1.3e-4)
FSCALE = 256.0  # fp16 range lift for the conv filter rows

_CACHE = {}


def _build_crev() -> np.ndarray:
    """C_rev [8192, 1024] f32: column i equals C[:, 1022-i]; column 1023 is 0."""
    k = np.arange(W, dtype=np.float64)
    alpha = np.full(W, 2.0)
    alpha[0] = 1.0
    alpha[-1] = 1.0
    alpha /= NFFT
    j = np.arange(1023, dtype=np.float64)
    wj = 0.5 - 0.5 * np.cos(2.0 * np.pi * (j + 2.0) / 1024.0)
    ang = (2.0 * np.pi / NFFT) * np.outer(k, 7681.0 + j)
    C = (alpha[:, None] * np.cos(ang)) * wj[None, :]
    crev = np.zeros((W, IRP), dtype=np.float64)
    crev[:, :1023] = C[:, ::-1]
    return np.ascontiguousarray(crev, dtype=np.float32)


def _begin_program():
    import concourse.mybir as mybir
    import concourse.tile as tile
    from concourse import bacc

    nc = bacc.Bacc("TRN2", target_bir_lowering=False, debug=False, num_devices=NCORES)
    return nc, tile, mybir


def _build_x_aug(nc, const, vel, kk, f32):
    """x_aug [128, DA] = [vel, K/MAX_STEPS, 1]."""
    x_aug = const.tile([128, DA], f32)
    nc.sync.dma_start(x_aug[:, 0:D], vel)
    nc.sync.dma_start(x_aug[:, D : D + 1], kk)
    nc.scalar.mul(x_aug[:, D : D + 1], x_aug[:, D : D + 1], 1.0 / MAX_STEPS)
    nc.vector.memset(x_aug[:, D + 1 : DA], 1.0)
    return x_aug


def _build_prog1():
    """Per-core: mean head, local max of mean+eps, W_ic column slice."""
    from contextlib import ExitStack

    nc, tile, mybir = _begin_program()
    f32 = mybir.dt.float32
    f16 = mybir.dt.float16
    AFT = mybir.ActivationFunctionType
    X = mybir.AxisListType.X

    vel = nc.dram_tensor("vel", [BSH, D], f32, kind="ExternalInput").ap()
    kk = nc.dram_tensor("kk", [BSH, 1], f32, kind="ExternalInput").ap()
    eps_sh = nc.dram_tensor("eps_sh", [BSH, W], f32, kind="ExternalInput").ap()
    # host-prelaid SBUF layouts: wsb[p, DA*c + d] = W_aug^T[128c + p, d] (fp16)
    wsb_d = nc.dram_tensor("wsb", [128, 64 * DA], f16, kind="ExternalInput").ap()
    # csb[p, 128c + i] = CSCALE * C_rev[128c + p, my_slice + i] (fp16)
    csb_d = nc.dram_tensor("csb", [128, 64 * BSH], f16, kind="ExternalInput").ap()
    w_mean = nc.dram_tensor("w_mean", [DA, 1], f32, kind="ExternalInput").ap()
    mean_out = nc.dram_tensor("mean_out", [BSH, 1], f32, kind="ExternalOutput").ap()
    lmax_out = nc.dram_tensor("lmax_out", [1, 1], f32, kind="ExternalOutput").ap()
    wic_out = nc.dram_tensor("wic_out", [DA, BSH], f32, kind="ExternalOutput").ap()

    from concourse import masks

    NG = 8  # csb DMA groups for DMA/matmul overlap

    with tile.TileContext(nc) as tc, ExitStack() as ctx:
        const = ctx.enter_context(tc.tile_pool(name="const", bufs=1))
        work = ctx.enter_context(tc.tile_pool(name="work", bufs=2))
        pmisc = ctx.enter_context(tc.tile_pool(name="pmisc", bufs=2, space="PSUM"))
        pwic = ctx.enter_context(tc.tile_pool(name="pwic", bufs=1, space="PSUM"))

        # small critical loads first on the sync ring
        x_aug = _build_x_aug(nc, const, vel, kk, f32)
        wme = const.tile([DA, 1], f32)
        nc.sync.dma_start(wme[:], w_mean)

        # big streams on the scalar (ACT) HWDGE ring
        wsb = const.tile([128, 64 * DA], f16)
        nc.scalar.dma_start(wsb[:], wsb_d)
        csb = const.tile([128, 64 * BSH], f16)
        GW = 64 * BSH // NG
        for g in range(NG):
            nc.scalar.dma_start(
                csb[:, GW * g : GW * (g + 1)], csb_d[:, GW * g : GW * (g + 1)]
            )

        ident = const.tile([128, 128], f32)
        masks.make_identity(nc, ident[:])

        xT_ps = pmisc.tile([DA, 128], f32, tag="misc")
        nc.tensor.transpose(xT_ps[:], x_aug[:], ident[:])
        xT = const.tile([DA, 128], f32)
        nc.vector.tensor_copy(xT[:], xT_ps[:])

        mean_ps = pmisc.tile([128, 1], f32, tag="misc")
        nc.tensor.matmul(mean_ps[:], lhsT=xT[:], rhs=wme[:], start=True, stop=True)
        mean_sb = const.tile([128, 1], f32)
        nc.scalar.activation(mean_sb[:], mean_ps[:], AFT.Tanh)
        nc.sync.dma_start(mean_out, mean_sb[:])

        # local max of (mean_b + eps_b[w]) over this shard
        rm4 = const.tile([128, 4], f32)
        for i in range(4):
            ch = work.tile([128, 2048], f32, tag="epschunk")
            nc.sync.dma_start(ch[:], eps_sh[:, 2048 * i : 2048 * (i + 1)])
            nc.vector.reduce_max(rm4[:, i : i + 1], ch[:], axis=X)
        rm1 = const.tile([128, 1], f32)
        nc.vector.reduce_max(rm1[:], rm4[:], axis=X)
        nc.vector.tensor_add(rm1[:], rm1[:], mean_sb[:])
        rmT_ps = pmisc.tile([1, 128], f32, tag="misc")
        nc.tensor.transpose(rmT_ps[:], rm1[:], ident[:])
        lmax_sb = const.tile([1, 1], f32)
        nc.vector.reduce_max(lmax_sb[:], rmT_ps[:], axis=X)
        nc.sync.dma_start(lmax_out, lmax_sb[0:1, 0:1])

        # W_ic slice: [W_coeff; b_coeff] @ C_rev[:, my 128 cols], grouped to
        # overlap with the csb stream
        wic_ps = pwic.tile([DA, 128], f32, tag="wic")
        per_g = 64 // NG
        for c in range(64):
            nc.tensor.matmul(
                wic_ps[:],
                lhsT=wsb[:, DA * c : DA * (c + 1)],
                rhs=csb[:, BSH * c : BSH * (c + 1)],
                start=(c == 0),
                stop=(c == 63),
            )
        wic_sb = work.tile([DA, BSH], f32)
        nc.vector.tensor_scalar_mul(wic_sb[:], wic_ps[:], 1.0 / CSCALE)
        nc.sync.dma_start(wic_out, wic_sb[:])

    nc.compile()
    return nc


def _build_prog2():
    """Per-core: amps head, filter rows, Toeplitz-matmul convolution."""
    from contextlib import ExitStack

    import concourse.bass as bass

    nc, tile, mybir = _begin_program()
    f32 = mybir.dt.float32
    f32r = mybir.dt.float32r
    f16 = mybir.dt.float16
    AFT = mybir.ActivationFunctionType

    vel = nc.dram_tensor("vel", [BSH, D], f32, kind="ExternalInput").ap()
    kk = nc.dram_tensor("kk", [BSH, 1], f32, kind="ExternalInput").ap()
    vel0 = nc.dram_tensor("vel0", [1, D], f32, kind="ExternalInput").ap()
    k0 = nc.dram_tensor("k0", [1, 1], f32, kind="ExternalInput").ap()
    eps0 = nc.dram_tensor("eps0", [1, W], f32, kind="ExternalInput").ap()
    w_amps = nc.dram_tensor("w_amps", [DA, 1], f32, kind="ExternalInput").ap()
    w_mean = nc.dram_tensor("w_mean", [DA, 1], f32, kind="ExternalInput").ap()
    m_in = nc.dram_tensor("m_in", [1, 1], f32, kind="ExternalInput").ap()
    wic_full = nc.dram_tensor("wic_full", [DA, IRP], f32, kind="ExternalInput").ap()
    out_noise = nc.dram_tensor("out_noise", [BSH, W], f32, kind="ExternalOutput").ap()

    from concourse import masks

    with tile.TileContext(nc) as tc, ExitStack() as ctx:
        const = ctx.enter_context(tc.tile_pool(name="const", bufs=1))
        work = ctx.enter_context(tc.tile_pool(name="work", bufs=3))
        pmisc = ctx.enter_context(tc.tile_pool(name="pmisc", bufs=2, space="PSUM"))
        pconv = ctx.enter_context(tc.tile_pool(name="pconv", bufs=4, space="PSUM"))
        pft = ctx.enter_context(tc.tile_pool(name="pft", bufs=2, space="PSUM"))
        dram = ctx.enter_context(tc.tile_pool(name="dram", bufs=1, space="DRAM"))

        # ---- n0 / Toeplitz chain first (it gates the conv) ----
        eps02d = const.tile([128, 64], f32)
        nc.sync.dma_start(eps02d[:], eps0.rearrange("a (p q) -> (a p) q", p=128))

        x0T = const.tile([DA, 1], f32)
        nc.vector.memset(x0T[:], 1.0)
        nc.sync.dma_start(x0T[0:D, 0:1], vel0.rearrange("a b -> b a"))
        nc.sync.dma_start(x0T[D : D + 1, 0:1], k0)
        nc.scalar.mul(x0T[D : D + 1, :], x0T[D : D + 1, :], 1.0 / MAX_STEPS)

        wme = const.tile([DA, 1], f32)
        nc.scalar.dma_start(wme[:], w_mean)
        wam = const.tile([DA, 1], f32)
        nc.scalar.dma_start(wam[:], w_amps)
        gmax = const.tile([1, 1], f32)
        nc.scalar.dma_start(gmax[:], m_in)
        wic_sb = const.tile([DA, IRP], f32r)
        nc.scalar.dma_start(wic_sb[:], wic_full.bitcast(f32r))

        m0_ps = pmisc.tile([1, 1], f32, tag="misc")
        nc.tensor.matmul(m0_ps[:], lhsT=x0T[:], rhs=wme[:], start=True, stop=True)
        m0_sb = const.tile([1, 1], f32)
        nc.scalar.activation(m0_sb[:], m0_ps[:], AFT.Tanh)

        ones_row = const.tile([1, 128], f32)
        nc.vector.memset(ones_row[:], 1.0)
        m0bc_ps = pmisc.tile([128, 1], f32, tag="misc")
        nc.tensor.matmul(m0bc_ps[:], lhsT=ones_row[:], rhs=m0_sb[:], start=True, stop=True)
        m0bc = const.tile([128, 1], f32)
        nc.vector.tensor_copy(m0bc[:], m0bc_ps[:])

        n02d = const.tile([128, 64], f16)
        nc.vector.tensor_scalar_add(n02d[:], eps02d[:], m0bc[:])

        n0d = dram.tile([1, N0PAD], f16)
        zpad = const.tile([128, 4], f16)
        nc.vector.memset(zpad[:], 0.0)
        nc.sync.dma_start(n0d[0:1, 0:TPAD].rearrange("a (p q) -> (a p) q", p=128), zpad[:])
        nc.sync.dma_start(
            n0d[0:1, TPAD + W : N0PAD].rearrange("a (p q) -> (a p) q", p=128), zpad[:]
        )
        nc.sync.dma_start(
            n0d[0:1, TPAD : TPAD + W].rearrange("a (p q) -> (a p) q", p=128), n02d[:]
        )

        t_sb = const.tile([128, TW], f16)
        toe_src = bass.AP(n0d[:].tensor, 0, [[1, 128], [1, TW]])
        nc.scalar.dma_start(t_sb[:], toe_src)

        # ---- heads and scaled filter rows ----
        ident = const.tile([128, 128], f32)
        masks.make_identity(nc, ident[:])

        x_aug = _build_x_aug(nc, const, vel, kk, f32)

        xT_ps = pmisc.tile([DA, 128], f32, tag="misc")
        nc.tensor.transpose(xT_ps[:], x_aug[:], ident[:])
        xT = const.tile([DA, 128], f32)
        nc.vector.tensor_copy(xT[:], xT_ps[:])

        amps_ps = pmisc.tile([128, 1], f32, tag="misc")
        nc.tensor.matmul(amps_ps[:], lhsT=xT[:], rhs=wam[:], start=True, stop=True)
        amps_sb = const.tile([128, 1], f32)
        nc.scalar.activation(amps_sb[:], amps_ps[:], AFT.Sigmoid)

        # s_b = FSCALE * amps_b / M  (M broadcast via ones-matmul)
        gmax_ps = pmisc.tile([128, 1], f32, tag="misc")
        nc.tensor.matmul(gmax_ps[:], lhsT=ones_row[:], rhs=gmax[:], start=True, stop=True)
        minv = const.tile([128, 1], f32)
        nc.vector.reciprocal(minv[:], gmax_ps[:])
        s_sb = const.tile([128, 1], f32)
        nc.vector.tensor_mul(s_sb[:], amps_sb[:], minv[:])
        nc.scalar.mul(s_sb[:], s_sb[:], FSCALE)

        xs = const.tile([128, DA], f32)
        nc.vector.tensor_scalar_mul(xs[:], x_aug[:], s_sb[:])
        xsT_ps = pmisc.tile([DA, 128], f32, tag="misc")
        nc.tensor.transpose(xsT_ps[:], xs[:], ident[:])
        xsT = const.tile([DA, 128], f32r)
        nc.vector.tensor_copy(xsT[:], xsT_ps[:])

        # filter rows fT[i, b] = sum_d W_ic[d, i] * xs[b, d]  (fp16, x FSCALE)
        fT = const.tile([128, IRP], f16)
        for c in range(8):
            fp = pft.tile([128, 128], f32, tag="fp")
            nc.tensor.matmul(
                fp[:],
                lhsT=wic_sb[:, 128 * c : 128 * (c + 1)],
                rhs=xsT[:],
                start=True,
                stop=True,
            )
            nc.vector.tensor_copy(fT[:, 128 * c : 128 * (c + 1)], fp[:])

        # ---- conv: out[b, t] = 2^-8 sum_i fT[i, b] * T[i%128, t + 128*(i//128)]
        for t in range(16):
            po = pconv.tile([128, 512], f32, tag="conv")
            for c in range(8):
                nc.tensor.matmul(
                    po[:],
                    lhsT=fT[:, 128 * c : 128 * (c + 1)],
                    rhs=t_sb[:, 128 * c + 512 * t : 128 * c + 512 * t + 512],
                    start=(c == 0),
                    stop=(c == 7),
                )
            ob = work.tile([128, 512], f32, tag="outbounce")
            nc.vector.tensor_scalar_mul(ob[:], po[:], 1.0 / FSCALE)
            nc.sync.dma_start(out_noise[:, 512 * t : 512 * (t + 1)], ob[:])

    nc.compile()
    return nc


def _get_progs():
    if "nc1" not in _CACHE:
        _CACHE["nc1"] = _build_prog1()
        _CACHE["nc2"] = _build_prog2()
    return _CACHE["nc1"], _CACHE["nc2"]


def _get_crev16():
    """(CSCALE * C_rev) as fp16, chunk-relaid: [8192, 1024] -> [64, 128, 1024]."""
    if "crev16" not in _CACHE:
        crev = _build_crev()
        _CACHE["crev16"] = np.ascontiguousarray(
            (crev * CSCALE).astype(np.float16).reshape(64, 128, IRP)
        )
    return _CACHE["crev16"]


def _prep(inputs: dict) -> dict:
    p = {}
    p["vel"] = np.ascontiguousarray(np.asarray(inputs["vel_inputs"]), dtype=np.float32)
    p["K"] = np.ascontiguousarray(np.asarray(inputs["K"]), dtype=np.float32)
    p["eps"] = np.ascontiguousarray(np.asarray(inputs["eps"]), dtype=np.float32)
    w_coeff = np.asarray(inputs["W_coeff"], dtype=np.float32)
    b_coeff = np.asarray(inputs["b_coeff"], dtype=np.float32)
    w_aug_t = np.concatenate([w_coeff.T, b_coeff[:, None]], axis=1)  # [W, DA]
    # wsb[p, DA*c + d] = w_aug_t[128c + p, d], fp16
    p["wsb"] = np.ascontiguousarray(
        w_aug_t.reshape(64, 128, DA).transpose(1, 0, 2).reshape(128, 64 * DA)
    ).astype(np.float16)
    p["w_amps"] = np.ascontiguousarray(
        np.concatenate(
            [np.asarray(inputs["W_amps"], np.float32), np.asarray(inputs["b_amps"], np.float32)[:, None]],
            axis=0,
        )
    )
    p["w_mean"] = np.ascontiguousarray(
        np.concatenate(
            [np.asarray(inputs["W_mean"], np.float32), np.asarray(inputs["b_mean"], np.float32)[:, None]],
            axis=0,
        )
    )
    return p


def make_in_maps1(p: dict) -> list:
    crev16 = _get_crev16()  # [64, 128, IRP]
    maps = []
    for c in range(NCORES):
        csb = np.ascontiguousarray(
            crev16[:, :, BSH * c : BSH * (c + 1)]
            .transpose(1, 0, 2)
            .reshape(128, 64 * BSH)
        )
        sl = slice(BSH * c, BSH * (c + 1))
        maps.append(
            {
                "vel": np.ascontiguousarray(p["vel"][sl]),
                "kk": np.ascontiguousarray(p["K"][sl]),
                "eps_sh": np.ascontiguousarray(p["eps"][sl]),
                "wsb": p["wsb"],
                "csb": csb,
                "w_mean": p["w_mean"],
            }
        )
    return maps


def glue12(results1: list) -> tuple:
    """Host-side exchange: gather W_ic slices, max of shard maxima, mean row 0."""
    wic_full = np.ascontiguousarray(
        np.concatenate([r["wic_out"] for r in results1], axis=1)
    )  # [DA, 1024]
    m = np.max([r["lmax_out"][0, 0] for r in results1]).reshape(1, 1).astype(np.float32)
    mean = np.concatenate([r["mean_out"] for r in results1], axis=0)  # [B, 1]
    return wic_full, m, mean


def make_in_maps2(p: dict, wic_full: np.ndarray, m: np.ndarray) -> list:
    maps = []
    for c in range(NCORES):
        sl = slice(BSH * c, BSH * (c + 1))
        maps.append(
            {
                "vel": np.ascontiguousarray(p["vel"][sl]),
                "kk": np.ascontiguousarray(p["K"][sl]),
                "vel0": np.ascontiguousarray(p["vel"][0:1]),
                "k0": np.ascontiguousarray(p["K"][0:1]),
                "eps0": np.ascontiguousarray(p["eps"][0:1]),
                "w_amps": p["w_amps"],
                "w_mean": p["w_mean"],
                "m_in": m,
                "wic_full": wic_full,
            }
        )
    return maps


def kernel(**inputs):
    from concourse.bass_utils import run_bass_kernel_spmd

    nc1, nc2 = _get_progs()
    p = _prep(inputs)
    trace = os.environ.get("NOISE_KERNEL_TRACE", "0") == "1"
    core_ids = list(range(NCORES))

    res1 = run_bass_kernel_spmd(nc1, make_in_maps1(p), core_ids=core_ids, trace=trace)
    wic_full, m, mean = glue12(res1.results)
    res2 = run_bass_kernel_spmd(
        nc2, make_in_maps2(p, wic_full, m), core_ids=core_ids, trace=trace
    )
    _CACHE["last_result1"] = res1
    _CACHE["last_result2"] = res2
    out = np.concatenate([r["out_noise"] for r in res2.results], axis=0)
    return out, mean


# revision 18
# speedup vs baseline: 1.7119x; 1.0271x over previous
"""Trainium2 Bass kernel for nn_NoiseGenerator.

Math (verified against the jax reference on host):
  The reference's irfft -> fftshift -> hann-window -> slice pipeline is a fixed
  linear map of the 8192 spectral magnitudes into a 1023-tap impulse:
      impulse = noise_bands @ C,
      C[k, j] = w_j * alpha_k * cos(2*pi*k*(7681 + j) / 16382)
      w_j = 0.5 - 0.5*cos(2*pi*(j+2)/1024),  alpha = [1, 2, ..., 2, 1] / 16382
  The fft_convolve + crop is then an ordinary linear convolution of noise row 0
  with each batch row's impulse:
      out[b, t] = amps_b / M * sum_i f[b, i] * n0u[t - 512 + i]
      f[b, i] = impulse[b, 1022 - i] (reversal folded into C), n0u = mean_0 + eps[0]
      M = max(mean + eps)  (global max over the full batch)

Distribution over 8 cores: data-parallel over batch (128 rows/core), two
phases with host-mediated exchange (no device collectives -- measured ~100us
of barrier/CC overhead for tiny payloads on this fabric):
  phase 1 (per core): MLP mean head, shard-local max of mean+eps, and a
    128-column slice of W_ic = [W_coeff; b_coeff] @ C_rev (C column-sharded,
    fp16 with a 2^13 scale, unscaled on-device after the accumulation).
  host: concatenates the 8 W_ic slices (pure gather), takes max of the 8
    shard maxima (8 floats), picks mean[0] from core 0's output.
  phase 2 (per core): amps head, filter rows fT = W_ic^T x_scaled^T with
    amps/M (and a 2^8 fp16-headroom scale) folded in, Toeplitz-matmul
    convolution against fp16 shifted windows of noise row 0, 2^-8 applied on
    the PSUM drain, streamed to the output.
"""

import os

import numpy as np

B = 1024
D = 64
DA = D + 2  # vel feats + K/MAX_STEPS + const-1 (folds the coeff bias)
W = 8192
NFFT = 16382
IRP = 1024  # impulse taps padded 1023 -> 1024
NCORES = 8
BSH = B // NCORES
MAX_STEPS = 2799.0
TPAD = 512  # left zero-padding of n0
N0PAD = 9216  # 512 + 8192 + 512 zeros
TW = 9088  # Toeplitz tile free size: 8192 + 7*128 + 512 - 512
CSCALE = 8192.0  # fp16 range lift for C (|C| <= 1.3e-4)
FSCALE = 256.0  # fp16 range lift for the conv filter rows

_CACHE = {}


def _build_crev() -> np.ndarray:
    """C_rev [8192, 1024] f32: column i equals C[:, 1022-i]; column 1023 is 0."""
    k = np.arange(W, dtype=np.float64)
    alpha = np.full(W, 2.0)
    alpha[0] = 1.0
    alpha[-1] = 1.0
    alpha /= NFFT
    j = np.arange(1023, dtype=np.float64)
    wj = 0.5 - 0.5 * np.cos(2.0 * np.pi * (j + 2.0) / 1024.0)
    ang = (2.0 * np.pi / NFFT) * np.outer(k, 7681.0 + j)
    C = (alpha[:, None] * np.cos(ang)) * wj[None, :]
    crev = np.zeros((W, IRP), dtype=np.float64)
    crev[:, :1023] = C[:, ::-1]
    return np.ascontiguousarray(crev, dtype=np.float32)


def _begin_program():
    import concourse.mybir as mybir
    import concourse.tile as tile
    from concourse import bacc

    nc = bacc.Bacc("TRN2", target_bir_lowering=False, debug=False, num_devices=NCORES)
    return nc, tile, mybir


def _build_x_aug(nc, const, vel, kk, f32):
    """x_aug [128, DA] = [vel, K/MAX_STEPS, 1]."""
    x_aug = const.tile([128, DA], f32)
    nc.sync.dma_start(x_aug[:, 0:D], vel)
    nc.sync.dma_start(x_aug[:, D : D + 1], kk)
    nc.scalar.mul(x_aug[:, D : D + 1], x_aug[:, D : D + 1], 1.0 / MAX_STEPS)
    nc.vector.memset(x_aug[:, D + 1 : DA], 1.0)
    return x_aug


def _build_prog1():
    """Per-core: mean head, local max of mean+eps, W_ic column slice."""
    from contextlib import ExitStack

    nc, tile, mybir = _begin_program()
    f32 = mybir.dt.float32
    f16 = mybir.dt.float16
    AFT = mybir.ActivationFunctionType
    X = mybir.AxisListType.X

    vel = nc.dram_tensor("vel", [BSH, D], f32, kind="ExternalInput").ap()
    kk = nc.dram_tensor("kk", [BSH, 1], f32, kind="ExternalInput").ap()
    eps_sh = nc.dram_tensor("eps_sh", [BSH, W], f32, kind="ExternalInput").ap()
    # host-prelaid SBUF layouts: wsb[p, DA*c + d] = W_aug^T[128c + p, d] (fp16)
    wsb_d = nc.dram_tensor("wsb", [128, 64 * DA], f16, kind="ExternalInput").ap()
    # csb[p, 128c + i] = CSCALE * C_rev[128c + p, my_slice + i] (fp16)
    csb_d = nc.dram_tensor("csb", [128, 64 * BSH], f16, kind="ExternalInput").ap()
    w_mean = nc.dram_tensor("w_mean", [DA, 1], f32, kind="ExternalInput").ap()
    ident_in = nc.dram_tensor("ident", [128, 128], f32, kind="ExternalInput").ap()
    mean_out = nc.dram_tensor("mean_out", [BSH, 1], f32, kind="ExternalOutput").ap()
    lmax_out = nc.dram_tensor("lmax_out", [1, 1], f32, kind="ExternalOutput").ap()
    wic_out = nc.dram_tensor("wic_out", [DA, BSH], f32, kind="ExternalOutput").ap()

    NG = 8  # csb DMA groups for DMA/matmul overlap

    with tile.TileContext(nc) as tc, ExitStack() as ctx:
        const = ctx.enter_context(tc.tile_pool(name="const", bufs=1))
        work = ctx.enter_context(tc.tile_pool(name="work", bufs=4))
        pmisc = ctx.enter_context(tc.tile_pool(name="pmisc", bufs=2, space="PSUM"))
        pwic = ctx.enter_context(tc.tile_pool(name="pwic", bufs=1, space="PSUM"))

        # small critical loads first on the sync ring
        x_aug = _build_x_aug(nc, const, vel, kk, f32)
        wme = const.tile([DA, 1], f32)
        nc.sync.dma_start(wme[:], w_mean)

        # big streams on the scalar (ACT) HWDGE ring
        wsb = const.tile([128, 64 * DA], f16)
        nc.scalar.dma_start(wsb[:], wsb_d)
        csb = const.tile([128, 64 * BSH], f16)
        GW = 64 * BSH // NG
        for g in range(NG):
            nc.scalar.dma_start(
                csb[:, GW * g : GW * (g + 1)], csb_d[:, GW * g : GW * (g + 1)]
            )

        ident = const.tile([128, 128], f32)
        nc.scalar.dma_start(ident[:], ident_in)

        xT_ps = pmisc.tile([DA, 128], f32, tag="misc")
        nc.tensor.transpose(xT_ps[:], x_aug[:], ident[:])
        xT = const.tile([DA, 128], f32)
        nc.vector.tensor_copy(xT[:], xT_ps[:])

        mean_ps = pmisc.tile([128, 1], f32, tag="misc")
        nc.tensor.matmul(mean_ps[:], lhsT=xT[:], rhs=wme[:], start=True, stop=True)
        mean_sb = const.tile([128, 1], f32)
        nc.scalar.activation(mean_sb[:], mean_ps[:], AFT.Tanh)
        nc.sync.dma_start(mean_out, mean_sb[:])

        # local max of (mean_b + eps_b[w]) over this shard
        rm4 = const.tile([128, 4], f32)
        for i in range(4):
            ch = work.tile([128, 2048], f32, tag="epschunk")
            nc.sync.dma_start(ch[:], eps_sh[:, 2048 * i : 2048 * (i + 1)])
            nc.vector.reduce_max(rm4[:, i : i + 1], ch[:], axis=X)
        rm1 = const.tile([128, 1], f32)
        nc.vector.reduce_max(rm1[:], rm4[:], axis=X)
        nc.vector.tensor_add(rm1[:], rm1[:], mean_sb[:])
        rmT_ps = pmisc.tile([1, 128], f32, tag="misc")
        nc.tensor.transpose(rmT_ps[:], rm1[:], ident[:])
        lmax_sb = const.tile([1, 1], f32)
        nc.vector.reduce_max(lmax_sb[:], rmT_ps[:], axis=X)
        nc.sync.dma_start(lmax_out, lmax_sb[0:1, 0:1])

        # W_ic slice: [W_coeff; b_coeff] @ C_rev[:, my 128 cols], grouped to
        # overlap with the csb stream
        wic_ps = pwic.tile([DA, 128], f32, tag="wic")
        for c in range(64):
            nc.tensor.matmul(
                wic_ps[:],
                lhsT=wsb[:, DA * c : DA * (c + 1)],
                rhs=csb[:, BSH * c : BSH * (c + 1)],
                start=(c == 0),
                stop=(c == 63),
            )
        wic_sb = work.tile([DA, BSH], f32)
        nc.vector.tensor_scalar_mul(wic_sb[:], wic_ps[:], 1.0 / CSCALE)
        nc.sync.dma_start(wic_out, wic_sb[:])

    nc.compile()
    return nc


def _build_prog2():
    """Per-core: amps head, filter rows, Toeplitz-matmul convolution."""
    from contextlib import ExitStack

    import concourse.bass as bass

    nc, tile, mybir = _begin_program()
    f32 = mybir.dt.float32
    f32r = mybir.dt.float32r
    f16 = mybir.dt.float16
    AFT = mybir.ActivationFunctionType

    vel = nc.dram_tensor("vel", [BSH, D], f32, kind="ExternalInput").ap()
    kk = nc.dram_tensor("kk", [BSH, 1], f32, kind="ExternalInput").ap()
    vel0 = nc.dram_tensor("vel0", [1, D], f32, kind="ExternalInput").ap()
    k0 = nc.dram_tensor("k0", [1, 1], f32, kind="ExternalInput").ap()
    eps0 = nc.dram_tensor("eps0", [1, W], f32, kind="ExternalInput").ap()
    w_amps = nc.dram_tensor("w_amps", [DA, 1], f32, kind="ExternalInput").ap()
    w_mean = nc.dram_tensor("w_mean", [DA, 1], f32, kind="ExternalInput").ap()
    m_in = nc.dram_tensor("m_in", [1, 1], f32, kind="ExternalInput").ap()
    wic_full = nc.dram_tensor("wic_full", [DA, IRP], f32, kind="ExternalInput").ap()
    ident_in = nc.dram_tensor("ident", [128, 128], f32, kind="ExternalInput").ap()
    out_noise = nc.dram_tensor("out_noise", [BSH, W], f32, kind="ExternalOutput").ap()

    with tile.TileContext(nc) as tc, ExitStack() as ctx:
        const = ctx.enter_context(tc.tile_pool(name="const", bufs=1))
        work = ctx.enter_context(tc.tile_pool(name="work", bufs=3))
        pmisc = ctx.enter_context(tc.tile_pool(name="pmisc", bufs=2, space="PSUM"))
        pconv = ctx.enter_context(tc.tile_pool(name="pconv", bufs=4, space="PSUM"))
        pft = ctx.enter_context(tc.tile_pool(name="pft", bufs=2, space="PSUM"))
        dram = ctx.enter_context(tc.tile_pool(name="dram", bufs=1, space="DRAM"))

        # ---- n0 / Toeplitz chain first (it gates the conv) ----
        eps02d = const.tile([128, 64], f32)
        nc.sync.dma_start(eps02d[:], eps0.rearrange("a (p q) -> (a p) q", p=128))

        x0T = const.tile([DA, 1], f32)
        nc.vector.memset(x0T[:], 1.0)
        nc.sync.dma_start(x0T[0:D, 0:1], vel0.rearrange("a b -> b a"))
        nc.sync.dma_start(x0T[D : D + 1, 0:1], k0)
        nc.scalar.mul(x0T[D : D + 1, :], x0T[D : D + 1, :], 1.0 / MAX_STEPS)

        wme = const.tile([DA, 1], f32)
        nc.scalar.dma_start(wme[:], w_mean)
        wam = const.tile([DA, 1], f32)
        nc.scalar.dma_start(wam[:], w_amps)
        gmax = const.tile([1, 1], f32)
        nc.scalar.dma_start(gmax[:], m_in)
        wic_sb = const.tile([DA, IRP], f32r)
        nc.scalar.dma_start(wic_sb[:], wic_full.bitcast(f32r))

        m0_ps = pmisc.tile([1, 1], f32, tag="misc")
        nc.tensor.matmul(m0_ps[:], lhsT=x0T[:], rhs=wme[:], start=True, stop=True)
        m0_sb = const.tile([1, 1], f32)
        nc.scalar.activation(m0_sb[:], m0_ps[:], AFT.Tanh)

        ones_row = const.tile([1, 128], f32)
        nc.vector.memset(ones_row[:], 1.0)
        m0bc_ps = pmisc.tile([128, 1], f32, tag="misc")
        nc.tensor.matmul(m0bc_ps[:], lhsT=ones_row[:], rhs=m0_sb[:], start=True, stop=True)
        m0bc = const.tile([128, 1], f32)
        nc.vector.tensor_copy(m0bc[:], m0bc_ps[:])

        n02d = const.tile([128, 64], f16)
        nc.vector.tensor_scalar_add(n02d[:], eps02d[:], m0bc[:])

        n0d = dram.tile([1, N0PAD], f16)
        zpad = const.tile([128, 4], f16)
        nc.vector.memset(zpad[:], 0.0)
        nc.sync.dma_start(n0d[0:1, 0:TPAD].rearrange("a (p q) -> (a p) q", p=128), zpad[:])
        nc.sync.dma_start(
            n0d[0:1, TPAD + W : N0PAD].rearrange("a (p q) -> (a p) q", p=128), zpad[:]
        )
        nc.sync.dma_start(
            n0d[0:1, TPAD : TPAD + W].rearrange("a (p q) -> (a p) q", p=128), n02d[:]
        )

        t_sb = const.tile([128, TW], f16)
        TH = TW // 2
        toe_a = bass.AP(n0d[:].tensor, 0, [[1, 128], [1, TH]])
        toe_b = bass.AP(n0d[:].tensor, TH, [[1, 128], [1, TW - TH]])
        nc.scalar.dma_start(t_sb[:, 0:TH], toe_a)
        nc.sync.dma_start(t_sb[:, TH:TW], toe_b)

        # ---- heads and scaled filter rows ----
        ident = const.tile([128, 128], f32)
        nc.scalar.dma_start(ident[:], ident_in)

        x_aug = _build_x_aug(nc, const, vel, kk, f32)

        xT_ps = pmisc.tile([DA, 128], f32, tag="misc")
        nc.tensor.transpose(xT_ps[:], x_aug[:], ident[:])
        xT = const.tile([DA, 128], f32)
        nc.vector.tensor_copy(xT[:], xT_ps[:])

        amps_ps = pmisc.tile([128, 1], f32, tag="misc")
        nc.tensor.matmul(amps_ps[:], lhsT=xT[:], rhs=wam[:], start=True, stop=True)
        amps_sb = const.tile([128, 1], f32)
        nc.scalar.activation(amps_sb[:], amps_ps[:], AFT.Sigmoid)

        # s_b = FSCALE * amps_b / M  (M broadcast via ones-matmul)
        gmax_ps = pmisc.tile([128, 1], f32, tag="misc")
        nc.tensor.matmul(gmax_ps[:], lhsT=ones_row[:], rhs=gmax[:], start=True, stop=True)
        minv = const.tile([128, 1], f32)
        nc.vector.reciprocal(minv[:], gmax_ps[:])
        s_sb = const.tile([128, 1], f32)
        nc.vector.tensor_mul(s_sb[:], amps_sb[:], minv[:])
        nc.scalar.mul(s_sb[:], s_sb[:], FSCALE)

        xs = const.tile([128, DA], f32)
        nc.vector.tensor_scalar_mul(xs[:], x_aug[:], s_sb[:])
        xsT_ps = pmisc.tile([DA, 128], f32, tag="misc")
        nc.tensor.transpose(xsT_ps[:], xs[:], ident[:])
        xsT = const.tile([DA, 128], f32r)
        nc.vector.tensor_copy(xsT[:], xsT_ps[:])

        # filter rows fT[i, b] = sum_d W_ic[d, i] * xs[b, d]  (fp16, x FSCALE)
        fT = const.tile([128, IRP], f16)
        for c in range(8):
            fp = pft.tile([128, 128], f32, tag="fp")
            nc.tensor.matmul(
                fp[:],
                lhsT=wic_sb[:, 128 * c : 128 * (c + 1)],
                rhs=xsT[:],
                start=True,
                stop=True,
            )
            nc.vector.tensor_copy(fT[:, 128 * c : 128 * (c + 1)], fp[:])

        # ---- conv: out[b, t] = 2^-8 sum_i fT[i, b] * T[i%128, t + 128*(i//128)]
        for t in range(16):
            po = pconv.tile([128, 512], f32, tag="conv")
            for c in range(8):
                nc.tensor.matmul(
                    po[:],
                    lhsT=fT[:, 128 * c : 128 * (c + 1)],
                    rhs=t_sb[:, 128 * c + 512 * t : 128 * c + 512 * t + 512],
                    start=(c == 0),
                    stop=(c == 7),
                )
            ob = work.tile([128, 512], f32, tag="outbounce")
            nc.vector.tensor_scalar_mul(ob[:], po[:], 1.0 / FSCALE)
            nc.sync.dma_start(out_noise[:, 512 * t : 512 * (t + 1)], ob[:])

    nc.compile()
    return nc


def _get_progs():
    if "nc1" not in _CACHE:
        _CACHE["nc1"] = _build_prog1()
        _CACHE["nc2"] = _build_prog2()
    return _CACHE["nc1"], _CACHE["nc2"]


def _get_crev16():
    """(CSCALE * C_rev) as fp16, chunk-relaid: [8192, 1024] -> [64, 128, 1024]."""
    if "crev16" not in _CACHE:
        crev = _build_crev()
        _CACHE["crev16"] = np.ascontiguousarray(
            (crev * CSCALE).astype(np.float16).reshape(64, 128, IRP)
        )
    return _CACHE["crev16"]


def _prep(inputs: dict) -> dict:
    p = {}
    if "ident" not in _CACHE:
        _CACHE["ident"] = np.ascontiguousarray(np.eye(128, dtype=np.float32))
    p["ident"] = _CACHE["ident"]
    p["vel"] = np.ascontiguousarray(np.asarray(inputs["vel_inputs"]), dtype=np.float32)
    p["K"] = np.ascontiguousarray(np.asarray(inputs["K"]), dtype=np.float32)
    p["eps"] = np.ascontiguousarray(np.asarray(inputs["eps"]), dtype=np.float32)
    w_coeff = np.asarray(inputs["W_coeff"], dtype=np.float32)
    b_coeff = np.asarray(inputs["b_coeff"], dtype=np.float32)
    w_aug_t = np.concatenate([w_coeff.T, b_coeff[:, None]], axis=1)  # [W, DA]
    # wsb[p, DA*c + d] = w_aug_t[128c + p, d], fp16
    p["wsb"] = np.ascontiguousarray(
        w_aug_t.reshape(64, 128, DA).transpose(1, 0, 2).reshape(128, 64 * DA)
    ).astype(np.float16)
    p["w_amps"] = np.ascontiguousarray(
        np.concatenate(
            [np.asarray(inputs["W_amps"], np.float32), np.asarray(inputs["b_amps"], np.float32)[:, None]],
            axis=0,
        )
    )
    p["w_mean"] = np.ascontiguousarray(
        np.concatenate(
            [np.asarray(inputs["W_mean"], np.float32), np.asarray(inputs["b_mean"], np.float32)[:, None]],
            axis=0,
        )
    )
    return p


def make_in_maps1(p: dict) -> list:
    crev16 = _get_crev16()  # [64, 128, IRP]
    maps = []
    for c in range(NCORES):
        csb = np.ascontiguousarray(
            crev16[:, :, BSH * c : BSH * (c + 1)]
            .transpose(1, 0, 2)
            .reshape(128, 64 * BSH)
        )
        sl = slice(BSH * c, BSH * (c + 1))
        maps.append(
            {
                "vel": np.ascontiguousarray(p["vel"][sl]),
                "kk": np.ascontiguousarray(p["K"][sl]),
                "eps_sh": np.ascontiguousarray(p["eps"][sl]),
                "wsb": p["wsb"],
                "csb": csb,
                "w_mean": p["w_mean"],
                "ident": p["ident"],
            }
        )
    return maps


def glue12(results1: list) -> tuple:
    """Host-side exchange: gather W_ic slices, max of shard maxima, mean row 0."""
    wic_full = np.ascontiguousarray(
        np.concatenate([r["wic_out"] for r in results1], axis=1)
    )  # [DA, 1024]
    m = np.max([r["lmax_out"][0, 0] for r in results1]).reshape(1, 1).astype(np.float32)
    mean = np.concatenate([r["mean_out"] for r in results1], axis=0)  # [B, 1]
    return wic_full, m, mean


def make_in_maps2(p: dict, wic_full: np.ndarray, m: np.ndarray) -> list:
    maps = []
    for c in range(NCORES):
        sl = slice(BSH * c, BSH * (c + 1))
        maps.append(
            {
                "vel": np.ascontiguousarray(p["vel"][sl]),
                "kk": np.ascontiguousarray(p["K"][sl]),
                "vel0": np.ascontiguousarray(p["vel"][0:1]),
                "k0": np.ascontiguousarray(p["K"][0:1]),
                "eps0": np.ascontiguousarray(p["eps"][0:1]),
                "w_amps": p["w_amps"],
                "w_mean": p["w_mean"],
                "m_in": m,
                "wic_full": wic_full,
                "ident": p["ident"],
            }
        )
    return maps


def kernel(**inputs):
    from concourse.bass_utils import run_bass_kernel_spmd

    nc1, nc2 = _get_progs()
    p = _prep(inputs)
    trace = os.environ.get("NOISE_KERNEL_TRACE", "0") == "1"
    core_ids = list(range(NCORES))

    res1 = run_bass_kernel_spmd(nc1, make_in_maps1(p), core_ids=core_ids, trace=trace)
    wic_full, m, mean = glue12(res1.results)
    res2 = run_bass_kernel_spmd(
        nc2, make_in_maps2(p, wic_full, m), core_ids=core_ids, trace=trace
    )
    _CACHE["last_result1"] = res1
    _CACHE["last_result2"] = res2
    out = np.concatenate([r["out_noise"] for r in res2.results], axis=0)
    return out, mean


# revision 21
# speedup vs baseline: 1.7305x; 1.0109x over previous
"""Trainium2 Bass kernel for nn_NoiseGenerator.

Math (verified against the jax reference on host):
  The reference's irfft -> fftshift -> hann-window -> slice pipeline is a fixed
  linear map of the 8192 spectral magnitudes into a 1023-tap impulse:
      impulse = noise_bands @ C,
      C[k, j] = w_j * alpha_k * cos(2*pi*k*(7681 + j) / 16382)
      w_j = 0.5 - 0.5*cos(2*pi*(j+2)/1024),  alpha = [1, 2, ..., 2, 1] / 16382
  The fft_convolve + crop is then an ordinary linear convolution of noise row 0
  with each batch row's impulse:
      out[b, t] = amps_b / M * sum_i f[b, i] * n0u[t - 512 + i]
      f[b, i] = impulse[b, 1022 - i] (reversal folded into C), n0u = mean_0 + eps[0]
      M = max(mean + eps)  (global max over the full batch)

Distribution over 8 cores: data-parallel over batch (128 rows/core), two
phases with host-mediated exchange (no device collectives -- measured ~100us
of barrier/CC overhead for tiny payloads on this fabric):
  phase 1 (per core): MLP mean head, shard-local max of mean+eps, and a
    128-column slice of W_ic = [W_coeff; b_coeff] @ C_rev (C column-sharded,
    fp16 with a 2^13 scale, unscaled on-device after the accumulation).
  host: concatenates the 8 W_ic slices (pure gather), takes max of the 8
    shard maxima (8 floats), picks mean[0] from core 0's output.
  phase 2 (per core): amps head, filter rows fT = W_ic^T x_scaled^T with
    amps/M (and a 2^8 fp16-headroom scale) folded in, Toeplitz-matmul
    convolution against fp16 shifted windows of noise row 0, 2^-8 applied on
    the PSUM drain, streamed to the output.
"""

import os

import numpy as np

B = 1024
D = 64
DA = D + 2  # vel feats + K/MAX_STEPS + const-1 (folds the coeff bias)
W = 8192
NFFT = 16382
IRP = 1024  # impulse taps padded 1023 -> 1024
NCORES = 8
BSH = B // NCORES
MAX_STEPS = 2799.0
TPAD = 512  # left zero-padding of n0
N0PAD = 9216  # 512 + 8192 + 512 zeros
TW = 9088  # Toeplitz tile free size: 8192 + 7*128 + 512 - 512
CSCALE = 8192.0  # fp16 range lift for C (|C| <= 1.3e-4)
FSCALE = 256.0  # fp16 range lift for the conv filter rows

_CACHE = {}


def _build_crev() -> np.ndarray:
    """C_rev [8192, 1024] f32: column i equals C[:, 1022-i]; column 1023 is 0."""
    k = np.arange(W, dtype=np.float64)
    alpha = np.full(W, 2.0)
    alpha[0] = 1.0
    alpha[-1] = 1.0
    alpha /= NFFT
    j = np.arange(1023, dtype=np.float64)
    wj = 0.5 - 0.5 * np.cos(2.0 * np.pi * (j + 2.0) / 1024.0)
    ang = (2.0 * np.pi / NFFT) * np.outer(k, 7681.0 + j)
    C = (alpha[:, None] * np.cos(ang)) * wj[None, :]
    crev = np.zeros((W, IRP), dtype=np.float64)
    crev[:, :1023] = C[:, ::-1]
    return np.ascontiguousarray(crev, dtype=np.float32)


def _begin_program():
    import concourse.mybir as mybir
    import concourse.tile as tile
    from concourse import bacc

    nc = bacc.Bacc("TRN2", target_bir_lowering=False, debug=False, num_devices=NCORES)
    return nc, tile, mybir


def _build_x_aug(nc, const, vel, kk, f32):
    """x_aug [128, DA] = [vel, K/MAX_STEPS, 1]."""
    x_aug = const.tile([128, DA], f32)
    nc.sync.dma_start(x_aug[:, 0:D], vel)
    nc.sync.dma_start(x_aug[:, D : D + 1], kk)
    nc.scalar.mul(x_aug[:, D : D + 1], x_aug[:, D : D + 1], 1.0 / MAX_STEPS)
    nc.vector.memset(x_aug[:, D + 1 : DA], 1.0)
    return x_aug


def _build_prog1():
    """Per-core: mean head, local max of mean+eps, W_ic column slice."""
    from contextlib import ExitStack

    nc, tile, mybir = _begin_program()
    f32 = mybir.dt.float32
    f16 = mybir.dt.float16
    AFT = mybir.ActivationFunctionType
    X = mybir.AxisListType.X

    vel = nc.dram_tensor("vel", [BSH, D], f32, kind="ExternalInput").ap()
    kk = nc.dram_tensor("kk", [BSH, 1], f32, kind="ExternalInput").ap()
    eps_sh = nc.dram_tensor("eps_sh", [BSH, W], f32, kind="ExternalInput").ap()
    # host-prelaid SBUF layouts: wsb[p, DA*c + d] = W_aug^T[128c + p, d] (fp16)
    wsb_d = nc.dram_tensor("wsb", [128, 64 * DA], f16, kind="ExternalInput").ap()
    # csb[p, 128c + i] = CSCALE * C_rev[128c + p, my_slice + i] (fp16)
    csb_d = nc.dram_tensor("csb", [128, 64 * BSH], f16, kind="ExternalInput").ap()
    w_mean = nc.dram_tensor("w_mean", [DA, 1], f32, kind="ExternalInput").ap()
    w_amps = nc.dram_tensor("w_amps", [DA, 1], f32, kind="ExternalInput").ap()
    ident_in = nc.dram_tensor("ident", [128, 128], f32, kind="ExternalInput").ap()
    mean_out = nc.dram_tensor("mean_out", [BSH, 1], f32, kind="ExternalOutput").ap()
    amps_out = nc.dram_tensor("amps_out", [BSH, 1], f32, kind="ExternalOutput").ap()
    lmax_out = nc.dram_tensor("lmax_out", [1, 1], f32, kind="ExternalOutput").ap()
    wic_out = nc.dram_tensor("wic_out", [DA, BSH], f32, kind="ExternalOutput").ap()

    NG = 8  # csb DMA groups for DMA/matmul overlap

    with tile.TileContext(nc) as tc, ExitStack() as ctx:
        const = ctx.enter_context(tc.tile_pool(name="const", bufs=1))
        work = ctx.enter_context(tc.tile_pool(name="work", bufs=4))
        pmisc = ctx.enter_context(tc.tile_pool(name="pmisc", bufs=2, space="PSUM"))
        pwic = ctx.enter_context(tc.tile_pool(name="pwic", bufs=1, space="PSUM"))

        # small critical loads first on the sync ring
        x_aug = _build_x_aug(nc, const, vel, kk, f32)
        wme = const.tile([DA, 1], f32)
        nc.sync.dma_start(wme[:], w_mean)
        wam = const.tile([DA, 1], f32)
        nc.sync.dma_start(wam[:], w_amps)

        # big streams on the scalar (ACT) HWDGE ring, ident first (tiny)
        ident = const.tile([128, 128], f32)
        nc.scalar.dma_start(ident[:], ident_in)
        wsb = const.tile([128, 64 * DA], f16)
        csb = const.tile([128, 64 * BSH], f16)
        GW = 64 * BSH // NG
        GD = 64 * DA // NG
        for g in range(NG):
            nc.scalar.dma_start(
                csb[:, GW * g : GW * (g + 1)], csb_d[:, GW * g : GW * (g + 1)]
            )
            nc.scalar.dma_start(
                wsb[:, GD * g : GD * (g + 1)], wsb_d[:, GD * g : GD * (g + 1)]
            )

        xT_ps = pmisc.tile([DA, 128], f32, tag="misc")
        nc.tensor.transpose(xT_ps[:], x_aug[:], ident[:])
        xT = const.tile([DA, 128], f32)
        nc.vector.tensor_copy(xT[:], xT_ps[:])

        mean_ps = pmisc.tile([128, 1], f32, tag="misc")
        nc.tensor.matmul(mean_ps[:], lhsT=xT[:], rhs=wme[:], start=True, stop=True)
        mean_sb = const.tile([128, 1], f32)
        nc.scalar.activation(mean_sb[:], mean_ps[:], AFT.Tanh)
        nc.sync.dma_start(mean_out, mean_sb[:])

        amps_ps = pmisc.tile([128, 1], f32, tag="misc")
        nc.tensor.matmul(amps_ps[:], lhsT=xT[:], rhs=wam[:], start=True, stop=True)
        amps_sb = const.tile([128, 1], f32)
        nc.scalar.activation(amps_sb[:], amps_ps[:], AFT.Sigmoid)
        nc.sync.dma_start(amps_out, amps_sb[:])

        # local max of (mean_b + eps_b[w]) over this shard
        rm4 = const.tile([128, 4], f32)
        for i in range(4):
            ch = work.tile([128, 2048], f32, tag="epschunk")
            nc.sync.dma_start(ch[:], eps_sh[:, 2048 * i : 2048 * (i + 1)])
            nc.vector.reduce_max(rm4[:, i : i + 1], ch[:], axis=X)
        rm1 = const.tile([128, 1], f32)
        nc.vector.reduce_max(rm1[:], rm4[:], axis=X)
        nc.vector.tensor_add(rm1[:], rm1[:], mean_sb[:])
        rmT_ps = pmisc.tile([1, 128], f32, tag="misc")
        nc.tensor.transpose(rmT_ps[:], rm1[:], ident[:])
        lmax_sb = const.tile([1, 1], f32)
        nc.vector.reduce_max(lmax_sb[:], rmT_ps[:], axis=X)
        nc.sync.dma_start(lmax_out, lmax_sb[0:1, 0:1])

        # W_ic slice: [W_coeff; b_coeff] @ C_rev[:, my 128 cols], grouped to
        # overlap with the csb stream
        wic_ps = pwic.tile([DA, 128], f32, tag="wic")
        for c in range(64):
            nc.tensor.matmul(
                wic_ps[:],
                lhsT=wsb[:, DA * c : DA * (c + 1)],
                rhs=csb[:, BSH * c : BSH * (c + 1)],
                start=(c == 0),
                stop=(c == 63),
            )
        wic_sb = work.tile([DA, BSH], f32)
        nc.vector.tensor_scalar_mul(wic_sb[:], wic_ps[:], 1.0 / CSCALE)
        nc.sync.dma_start(wic_out, wic_sb[:])

    nc.compile()
    return nc


def _build_prog2():
    """Per-core: scaled filter rows, Toeplitz-matmul convolution."""
    from contextlib import ExitStack

    import concourse.bass as bass

    nc, tile, mybir = _begin_program()
    f32 = mybir.dt.float32
    f32r = mybir.dt.float32r
    f16 = mybir.dt.float16

    vel = nc.dram_tensor("vel", [BSH, D], f32, kind="ExternalInput").ap()
    kk = nc.dram_tensor("kk", [BSH, 1], f32, kind="ExternalInput").ap()
    eps0 = nc.dram_tensor("eps0", [1, W], f32, kind="ExternalInput").ap()
    amps_in = nc.dram_tensor("amps_in", [BSH, 1], f32, kind="ExternalInput").ap()
    mean0_in = nc.dram_tensor("mean0_in", [1, 1], f32, kind="ExternalInput").ap()
    m_in = nc.dram_tensor("m_in", [1, 1], f32, kind="ExternalInput").ap()
    wic_full = nc.dram_tensor("wic_full", [DA, IRP], f32, kind="ExternalInput").ap()
    ident_in = nc.dram_tensor("ident", [128, 128], f32, kind="ExternalInput").ap()
    out_noise = nc.dram_tensor("out_noise", [BSH, W], f32, kind="ExternalOutput").ap()

    with tile.TileContext(nc) as tc, ExitStack() as ctx:
        const = ctx.enter_context(tc.tile_pool(name="const", bufs=1))
        work = ctx.enter_context(tc.tile_pool(name="work", bufs=3))
        pmisc = ctx.enter_context(tc.tile_pool(name="pmisc", bufs=2, space="PSUM"))
        pconv = ctx.enter_context(tc.tile_pool(name="pconv", bufs=4, space="PSUM"))
        pft = ctx.enter_context(tc.tile_pool(name="pft", bufs=2, space="PSUM"))
        dram = ctx.enter_context(tc.tile_pool(name="dram", bufs=1, space="DRAM"))

        # ---- n0 / Toeplitz chain first (it gates the conv) ----
        # sync ring carries only this chain's loads up front
        m0_sb = const.tile([1, 1], f32)
        nc.sync.dma_start(m0_sb[:], mean0_in)
        eps02d = const.tile([128, 64], f32)
        nc.sync.dma_start(eps02d[:], eps0.rearrange("a (p q) -> (a p) q", p=128))
        amps_sb = const.tile([128, 1], f32)
        nc.sync.dma_start(amps_sb[:], amps_in)
        gmax = const.tile([1, 1], f32)
        nc.sync.dma_start(gmax[:], m_in)
        x_aug = _build_x_aug(nc, const, vel, kk, f32)

        ones_row = const.tile([1, 128], f32)
        nc.vector.memset(ones_row[:], 1.0)
        m0bc_ps = pmisc.tile([128, 1], f32, tag="misc")
        nc.tensor.matmul(m0bc_ps[:], lhsT=ones_row[:], rhs=m0_sb[:], start=True, stop=True)
        m0bc = const.tile([128, 1], f32)
        nc.vector.tensor_copy(m0bc[:], m0bc_ps[:])

        n02d = const.tile([128, 64], f16)
        nc.vector.tensor_scalar_add(n02d[:], eps02d[:], m0bc[:])

        n0d = dram.tile([1, N0PAD], f16)
        zpad = const.tile([128, 4], f16)
        nc.vector.memset(zpad[:], 0.0)
        nc.sync.dma_start(n0d[0:1, 0:TPAD].rearrange("a (p q) -> (a p) q", p=128), zpad[:])
        nc.sync.dma_start(
            n0d[0:1, TPAD + W : N0PAD].rearrange("a (p q) -> (a p) q", p=128), zpad[:]
        )
        nc.sync.dma_start(
            n0d[0:1, TPAD : TPAD + W].rearrange("a (p q) -> (a p) q", p=128), n02d[:]
        )

        # Toeplitz tile T[p, f] = n0pad[f + p]; 4 column pieces on two rings so
        # the conv can chase the stream
        t_sb = const.tile([128, TW], f16)
        NP = 4
        PW = TW // NP
        for k in range(NP):
            lo, hi = PW * k, PW * (k + 1) if k < NP - 1 else TW
            piece = bass.AP(n0d[:].tensor, lo, [[1, 128], [1, hi - lo]])
            eng = nc.sync if k % 2 == 0 else nc.gpsimd
            eng.dma_start(t_sb[:, lo:hi], piece)

        # ---- scalar ring: constants for the filter path; out stores later ----
        ident = const.tile([128, 128], f32)
        nc.scalar.dma_start(ident[:], ident_in)
        wic_sb = const.tile([DA, IRP], f32r)
        nc.scalar.dma_start(wic_sb[:], wic_full.bitcast(f32r))

        # s_b = FSCALE * amps_b / M  (M broadcast via ones-matmul)
        gmax_ps = pmisc.tile([128, 1], f32, tag="misc")
        nc.tensor.matmul(gmax_ps[:], lhsT=ones_row[:], rhs=gmax[:], start=True, stop=True)
        minv = const.tile([128, 1], f32)
        nc.vector.reciprocal(minv[:], gmax_ps[:])
        s_sb = const.tile([128, 1], f32)
        nc.vector.tensor_mul(s_sb[:], amps_sb[:], minv[:])
        nc.scalar.mul(s_sb[:], s_sb[:], FSCALE)

        xs = const.tile([128, DA], f32)
        nc.vector.tensor_scalar_mul(xs[:], x_aug[:], s_sb[:])
        xsT_ps = pmisc.tile([DA, 128], f32, tag="misc")
        nc.tensor.transpose(xsT_ps[:], xs[:], ident[:])
        xsT = const.tile([DA, 128], f32r)
        nc.vector.tensor_copy(xsT[:], xsT_ps[:])

        # filter rows fT[i, b] = sum_d W_ic[d, i] * xs[b, d]  (fp16, x FSCALE)
        fT = const.tile([128, IRP], f16)
        for c in range(8):
            fp = pft.tile([128, 128], f32, tag="fp")
            nc.tensor.matmul(
                fp[:],
                lhsT=wic_sb[:, 128 * c : 128 * (c + 1)],
                rhs=xsT[:],
                start=True,
                stop=True,
            )
            nc.vector.tensor_copy(fT[:, 128 * c : 128 * (c + 1)], fp[:])

        # ---- conv: out[b, t] = 2^-8 sum_i fT[i, b] * T[i%128, t + 128*(i//128)]
        for t in range(16):
            po = pconv.tile([128, 512], f32, tag="conv")
            for c in range(8):
                nc.tensor.matmul(
                    po[:],
                    lhsT=fT[:, 128 * c : 128 * (c + 1)],
                    rhs=t_sb[:, 128 * c + 512 * t : 128 * c + 512 * t + 512],
                    start=(c == 0),
                    stop=(c == 7),
                )
            ob = work.tile([128, 512], f32, tag="outbounce")
            nc.vector.tensor_scalar_mul(ob[:], po[:], 1.0 / FSCALE)
            nc.scalar.dma_start(out_noise[:, 512 * t : 512 * (t + 1)], ob[:])

    nc.compile()
    return nc


def _get_progs():
    if "nc1" not in _CACHE:
        _CACHE["nc1"] = _build_prog1()
        _CACHE["nc2"] = _build_prog2()
    return _CACHE["nc1"], _CACHE["nc2"]


def _get_crev16():
    """(CSCALE * C_rev) as fp16, chunk-relaid: [8192, 1024] -> [64, 128, 1024]."""
    if "crev16" not in _CACHE:
        crev = _build_crev()
        _CACHE["crev16"] = np.ascontiguousarray(
            (crev * CSCALE).astype(np.float16).reshape(64, 128, IRP)
        )
    return _CACHE["crev16"]


def _prep(inputs: dict) -> dict:
    p = {}
    if "ident" not in _CACHE:
        _CACHE["ident"] = np.ascontiguousarray(np.eye(128, dtype=np.float32))
    p["ident"] = _CACHE["ident"]
    p["vel"] = np.ascontiguousarray(np.asarray(inputs["vel_inputs"]), dtype=np.float32)
    p["K"] = np.ascontiguousarray(np.asarray(inputs["K"]), dtype=np.float32)
    p["eps"] = np.ascontiguousarray(np.asarray(inputs["eps"]), dtype=np.float32)
    w_coeff = np.asarray(inputs["W_coeff"], dtype=np.float32)
    b_coeff = np.asarray(inputs["b_coeff"], dtype=np.float32)
    w_aug_t = np.concatenate([w_coeff.T, b_coeff[:, None]], axis=1)  # [W, DA]
    # wsb[p, DA*c + d] = w_aug_t[128c + p, d], fp16
    p["wsb"] = np.ascontiguousarray(
        w_aug_t.reshape(64, 128, DA).transpose(1, 0, 2).reshape(128, 64 * DA)
    ).astype(np.float16)
    p["w_amps"] = np.ascontiguousarray(
        np.concatenate(
            [np.asarray(inputs["W_amps"], np.float32), np.asarray(inputs["b_amps"], np.float32)[:, None]],
            axis=0,
        )
    )
    p["w_mean"] = np.ascontiguousarray(
        np.concatenate(
            [np.asarray(inputs["W_mean"], np.float32), np.asarray(inputs["b_mean"], np.float32)[:, None]],
            axis=0,
        )
    )
    return p


def make_in_maps1(p: dict) -> list:
    crev16 = _get_crev16()  # [64, 128, IRP]
    maps = []
    for c in range(NCORES):
        csb = np.ascontiguousarray(
            crev16[:, :, BSH * c : BSH * (c + 1)]
            .transpose(1, 0, 2)
            .reshape(128, 64 * BSH)
        )
        sl = slice(BSH * c, BSH * (c + 1))
        maps.append(
            {
                "vel": np.ascontiguousarray(p["vel"][sl]),
                "kk": np.ascontiguousarray(p["K"][sl]),
                "eps_sh": np.ascontiguousarray(p["eps"][sl]),
                "wsb": p["wsb"],
                "csb": csb,
                "w_mean": p["w_mean"],
                "w_amps": p["w_amps"],
                "ident": p["ident"],
            }
        )
    return maps


def glue12(results1: list) -> tuple:
    """Host-side exchange: gather W_ic slices, max of shard maxima, mean row 0."""
    wic_full = np.ascontiguousarray(
        np.concatenate([r["wic_out"] for r in results1], axis=1)
    )  # [DA, 1024]
    m = np.max([r["lmax_out"][0, 0] for r in results1]).reshape(1, 1).astype(np.float32)
    mean = np.concatenate([r["mean_out"] for r in results1], axis=0)  # [B, 1]
    mean0 = np.ascontiguousarray(mean[0:1, 0:1])
    amps = [np.ascontiguousarray(r["amps_out"]) for r in results1]
    return wic_full, m, mean, mean0, amps


def make_in_maps2(
    p: dict, wic_full: np.ndarray, m: np.ndarray, mean0: np.ndarray, amps: list
) -> list:
    maps = []
    for c in range(NCORES):
        sl = slice(BSH * c, BSH * (c + 1))
        maps.append(
            {
                "vel": np.ascontiguousarray(p["vel"][sl]),
                "kk": np.ascontiguousarray(p["K"][sl]),
                "eps0": np.ascontiguousarray(p["eps"][0:1]),
                "amps_in": amps[c],
                "mean0_in": mean0,
                "m_in": m,
                "wic_full": wic_full,
                "ident": p["ident"],
            }
        )
    return maps


def kernel(**inputs):
    from concourse.bass_utils import run_bass_kernel_spmd

    nc1, nc2 = _get_progs()
    p = _prep(inputs)
    trace = os.environ.get("NOISE_KERNEL_TRACE", "0") == "1"
    core_ids = list(range(NCORES))

    res1 = run_bass_kernel_spmd(nc1, make_in_maps1(p), core_ids=core_ids, trace=trace)
    wic_full, m, mean, mean0, amps = glue12(res1.results)
    res2 = run_bass_kernel_spmd(
        nc2, make_in_maps2(p, wic_full, m, mean0, amps), core_ids=core_ids, trace=trace
    )
    _CACHE["last_result1"] = res1
    _CACHE["last_result2"] = res2
    out = np.concatenate([r["out_noise"] for r in res2.results], axis=0)
    return out, mean


# revision 22
# speedup vs baseline: 1.8498x; 1.0689x over previous
"""Trainium2 Bass kernel for nn_NoiseGenerator.

Math (verified against the jax reference on host):
  The reference's irfft -> fftshift -> hann-window -> slice pipeline is a fixed
  linear map of the 8192 spectral magnitudes into a 1023-tap impulse:
      impulse = noise_bands @ C,
      C[k, j] = w_j * alpha_k * cos(2*pi*k*(7681 + j) / 16382)
      w_j = 0.5 - 0.5*cos(2*pi*(j+2)/1024),  alpha = [1, 2, ..., 2, 1] / 16382
  The fft_convolve + crop is then an ordinary linear convolution of noise row 0
  with each batch row's impulse:
      out[b, t] = amps_b / M * sum_i f[b, i] * n0u[t - 512 + i]
      f[b, i] = impulse[b, 1022 - i] (reversal folded into C), n0u = mean_0 + eps[0]
      M = max(mean + eps)  (global max over the full batch)

Distribution over 8 cores: data-parallel over batch (128 rows/core), two
phases with host-mediated exchange (no device collectives -- measured ~100us
of barrier/CC overhead for tiny payloads on this fabric):
  phase 1 (per core): MLP mean head, shard-local max of mean+eps, and a
    128-column slice of W_ic = [W_coeff; b_coeff] @ C_rev (C column-sharded,
    fp16 with a 2^13 scale, unscaled on-device after the accumulation).
  host: concatenates the 8 W_ic slices (pure gather), takes max of the 8
    shard maxima (8 floats), picks mean[0] from core 0's output.
  phase 2 (per core): amps head, filter rows fT = W_ic^T x_scaled^T with
    amps/M (and a 2^8 fp16-headroom scale) folded in, Toeplitz-matmul
    convolution against fp16 shifted windows of noise row 0, 2^-8 applied on
    the PSUM drain, streamed to the output.
"""

import os

import numpy as np

B = 1024
D = 64
DA = D + 2  # vel feats + K/MAX_STEPS + const-1 (folds the coeff bias)
W = 8192
NFFT = 16382
IRP = 1024  # impulse taps padded 1023 -> 1024
NCORES = 8
BSH = B // NCORES
MAX_STEPS = 2799.0
TPAD = 512  # left zero-padding of n0
N0PAD = 9216  # 512 + 8192 + 512 zeros
TW = 9088  # Toeplitz tile free size: 8192 + 7*128 + 512 - 512
CSCALE = 8192.0  # fp16 range lift for C (|C| <= 1.3e-4)
FSCALE = 256.0  # fp16 range lift for the conv filter rows

_CACHE = {}


def _build_crev() -> np.ndarray:
    """C_rev [8192, 1024] f32: column i equals C[:, 1022-i]; column 1023 is 0."""
    k = np.arange(W, dtype=np.float64)
    alpha = np.full(W, 2.0)
    alpha[0] = 1.0
    alpha[-1] = 1.0
    alpha /= NFFT
    j = np.arange(1023, dtype=np.float64)
    wj = 0.5 - 0.5 * np.cos(2.0 * np.pi * (j + 2.0) / 1024.0)
    ang = (2.0 * np.pi / NFFT) * np.outer(k, 7681.0 + j)
    C = (alpha[:, None] * np.cos(ang)) * wj[None, :]
    crev = np.zeros((W, IRP), dtype=np.float64)
    crev[:, :1023] = C[:, ::-1]
    return np.ascontiguousarray(crev, dtype=np.float32)


def _begin_program():
    import concourse.mybir as mybir
    import concourse.tile as tile
    from concourse import bacc

    nc = bacc.Bacc("TRN2", target_bir_lowering=False, debug=False, num_devices=NCORES)
    return nc, tile, mybir


def _build_x_aug(nc, const, vel, kk, f32):
    """x_aug [128, DA] = [vel, K/MAX_STEPS, 1]."""
    x_aug = const.tile([128, DA], f32)
    nc.sync.dma_start(x_aug[:, 0:D], vel)
    nc.sync.dma_start(x_aug[:, D : D + 1], kk)
    nc.scalar.mul(x_aug[:, D : D + 1], x_aug[:, D : D + 1], 1.0 / MAX_STEPS)
    nc.vector.memset(x_aug[:, D + 1 : DA], 1.0)
    return x_aug


def _build_prog1():
    """Per-core: mean head, local max of mean+eps, W_ic column slice."""
    from contextlib import ExitStack

    nc, tile, mybir = _begin_program()
    f32 = mybir.dt.float32
    f16 = mybir.dt.float16
    AFT = mybir.ActivationFunctionType
    X = mybir.AxisListType.X

    vel = nc.dram_tensor("vel", [BSH, D], f32, kind="ExternalInput").ap()
    kk = nc.dram_tensor("kk", [BSH, 1], f32, kind="ExternalInput").ap()
    eps_sh = nc.dram_tensor("eps_sh", [BSH, W], f32, kind="ExternalInput").ap()
    # host-prelaid SBUF layouts: wsb[p, DA*c + d] = W_aug^T[128c + p, d] (fp16)
    wsb_d = nc.dram_tensor("wsb", [128, 64 * DA], f16, kind="ExternalInput").ap()
    # csb[p, 128c + i] = CSCALE * C_rev[128c + p, my_slice + i] (fp16)
    csb_d = nc.dram_tensor("csb", [128, 64 * BSH], f16, kind="ExternalInput").ap()
    w_mean = nc.dram_tensor("w_mean", [DA, 1], f32, kind="ExternalInput").ap()
    w_amps = nc.dram_tensor("w_amps", [DA, 1], f32, kind="ExternalInput").ap()
    ident_in = nc.dram_tensor("ident", [128, 128], f32, kind="ExternalInput").ap()
    mean_out = nc.dram_tensor("mean_out", [BSH, 1], f32, kind="ExternalOutput").ap()
    amps_out = nc.dram_tensor("amps_out", [BSH, 1], f32, kind="ExternalOutput").ap()
    lmax_out = nc.dram_tensor("lmax_out", [1, 1], f32, kind="ExternalOutput").ap()
    wic_out = nc.dram_tensor("wic_out", [DA, BSH], f32, kind="ExternalOutput").ap()

    NG = 8  # csb DMA groups for DMA/matmul overlap

    with tile.TileContext(nc) as tc, ExitStack() as ctx:
        const = ctx.enter_context(tc.tile_pool(name="const", bufs=1))
        work = ctx.enter_context(tc.tile_pool(name="work", bufs=4))
        pmisc = ctx.enter_context(tc.tile_pool(name="pmisc", bufs=2, space="PSUM"))
        pwic = ctx.enter_context(tc.tile_pool(name="pwic", bufs=1, space="PSUM"))

        # small critical loads first on the sync ring
        x_aug = _build_x_aug(nc, const, vel, kk, f32)
        wme = const.tile([DA, 1], f32)
        nc.sync.dma_start(wme[:], w_mean)
        wam = const.tile([DA, 1], f32)
        nc.sync.dma_start(wam[:], w_amps)

        # big streams on the scalar (ACT) HWDGE ring, ident first (tiny)
        ident = const.tile([128, 128], f32)
        nc.scalar.dma_start(ident[:], ident_in)
        wsb = const.tile([128, 64 * DA], f16)
        nc.scalar.dma_start(wsb[:], wsb_d)
        csb = const.tile([128, 64 * BSH], f16)
        GW = 64 * BSH // NG
        for g in range(NG):
            nc.scalar.dma_start(
                csb[:, GW * g : GW * (g + 1)], csb_d[:, GW * g : GW * (g + 1)]
            )

        xT_ps = pmisc.tile([DA, 128], f32, tag="misc")
        nc.tensor.transpose(xT_ps[:], x_aug[:], ident[:])
        xT = const.tile([DA, 128], f32)
        nc.vector.tensor_copy(xT[:], xT_ps[:])

        mean_ps = pmisc.tile([128, 1], f32, tag="misc")
        nc.tensor.matmul(mean_ps[:], lhsT=xT[:], rhs=wme[:], start=True, stop=True)
        mean_sb = const.tile([128, 1], f32)
        nc.scalar.activation(mean_sb[:], mean_ps[:], AFT.Tanh)
        nc.scalar.dma_start(mean_out, mean_sb[:])

        amps_ps = pmisc.tile([128, 1], f32, tag="misc")
        nc.tensor.matmul(amps_ps[:], lhsT=xT[:], rhs=wam[:], start=True, stop=True)
        amps_sb = const.tile([128, 1], f32)
        nc.scalar.activation(amps_sb[:], amps_ps[:], AFT.Sigmoid)
        nc.scalar.dma_start(amps_out, amps_sb[:])

        # local max of (mean_b + eps_b[w]) over this shard
        rm4 = const.tile([128, 4], f32)
        for i in range(4):
            ch = work.tile([128, 2048], f32, tag="epschunk")
            nc.sync.dma_start(ch[:], eps_sh[:, 2048 * i : 2048 * (i + 1)])
            nc.vector.reduce_max(rm4[:, i : i + 1], ch[:], axis=X)
        rm1 = const.tile([128, 1], f32)
        nc.vector.reduce_max(rm1[:], rm4[:], axis=X)
        nc.vector.tensor_add(rm1[:], rm1[:], mean_sb[:])
        rmT_ps = pmisc.tile([1, 128], f32, tag="misc")
        nc.tensor.transpose(rmT_ps[:], rm1[:], ident[:])
        lmax_sb = const.tile([1, 1], f32)
        nc.vector.reduce_max(lmax_sb[:], rmT_ps[:], axis=X)
        nc.scalar.dma_start(lmax_out, lmax_sb[0:1, 0:1])

        # W_ic slice: [W_coeff; b_coeff] @ C_rev[:, my 128 cols], grouped to
        # overlap with the csb stream
        wic_ps = pwic.tile([DA, 128], f32, tag="wic")
        for c in range(64):
            nc.tensor.matmul(
                wic_ps[:],
                lhsT=wsb[:, DA * c : DA * (c + 1)],
                rhs=csb[:, BSH * c : BSH * (c + 1)],
                start=(c == 0),
                stop=(c == 63),
            )
        wic_sb = work.tile([DA, BSH], f32)
        nc.vector.tensor_scalar_mul(wic_sb[:], wic_ps[:], 1.0 / CSCALE)
        nc.scalar.dma_start(wic_out, wic_sb[:])

    nc.compile()
    return nc


def _build_prog2():
    """Per-core: scaled filter rows, Toeplitz-matmul convolution."""
    from contextlib import ExitStack

    import concourse.bass as bass

    nc, tile, mybir = _begin_program()
    f32 = mybir.dt.float32
    f32r = mybir.dt.float32r
    f16 = mybir.dt.float16

    vel = nc.dram_tensor("vel", [BSH, D], f32, kind="ExternalInput").ap()
    kk = nc.dram_tensor("kk", [BSH, 1], f32, kind="ExternalInput").ap()
    eps0 = nc.dram_tensor("eps0", [1, W], f32, kind="ExternalInput").ap()
    amps_in = nc.dram_tensor("amps_in", [BSH, 1], f32, kind="ExternalInput").ap()
    mean0_in = nc.dram_tensor("mean0_in", [1, 1], f32, kind="ExternalInput").ap()
    m_in = nc.dram_tensor("m_in", [1, 1], f32, kind="ExternalInput").ap()
    wic_full = nc.dram_tensor("wic_full", [DA, IRP], f32, kind="ExternalInput").ap()
    ident_in = nc.dram_tensor("ident", [128, 128], f32, kind="ExternalInput").ap()
    out_noise = nc.dram_tensor("out_noise", [BSH, W], f32, kind="ExternalOutput").ap()

    with tile.TileContext(nc) as tc, ExitStack() as ctx:
        const = ctx.enter_context(tc.tile_pool(name="const", bufs=1))
        work = ctx.enter_context(tc.tile_pool(name="work", bufs=3))
        pmisc = ctx.enter_context(tc.tile_pool(name="pmisc", bufs=2, space="PSUM"))
        pconv = ctx.enter_context(tc.tile_pool(name="pconv", bufs=4, space="PSUM"))
        pft = ctx.enter_context(tc.tile_pool(name="pft", bufs=2, space="PSUM"))
        dram = ctx.enter_context(tc.tile_pool(name="dram", bufs=1, space="DRAM"))

        # ---- n0 / Toeplitz chain first (it gates the conv) ----
        # sync ring carries only this chain's loads up front
        m0_sb = const.tile([1, 1], f32)
        nc.sync.dma_start(m0_sb[:], mean0_in)
        eps02d = const.tile([128, 64], f32)
        nc.sync.dma_start(eps02d[:], eps0.rearrange("a (p q) -> (a p) q", p=128))
        amps_sb = const.tile([128, 1], f32)
        nc.sync.dma_start(amps_sb[:], amps_in)
        gmax = const.tile([1, 1], f32)
        nc.sync.dma_start(gmax[:], m_in)
        x_aug = _build_x_aug(nc, const, vel, kk, f32)

        ones_row = const.tile([1, 128], f32)
        nc.vector.memset(ones_row[:], 1.0)
        m0bc_ps = pmisc.tile([128, 1], f32, tag="misc")
        nc.tensor.matmul(m0bc_ps[:], lhsT=ones_row[:], rhs=m0_sb[:], start=True, stop=True)
        m0bc = const.tile([128, 1], f32)
        nc.vector.tensor_copy(m0bc[:], m0bc_ps[:])

        n02d = const.tile([128, 64], f16)
        nc.vector.tensor_scalar_add(n02d[:], eps02d[:], m0bc[:])

        n0d = dram.tile([1, N0PAD], f16)
        zpad = const.tile([128, 4], f16)
        nc.vector.memset(zpad[:], 0.0)
        nc.sync.dma_start(n0d[0:1, 0:TPAD].rearrange("a (p q) -> (a p) q", p=128), zpad[:])
        nc.sync.dma_start(
            n0d[0:1, TPAD + W : N0PAD].rearrange("a (p q) -> (a p) q", p=128), zpad[:]
        )
        nc.sync.dma_start(
            n0d[0:1, TPAD : TPAD + W].rearrange("a (p q) -> (a p) q", p=128), n02d[:]
        )

        # Toeplitz tile T[p, f] = n0pad[f + p]; 4 column pieces on two rings so
        # the conv can chase the stream
        t_sb = const.tile([128, TW], f16)
        NP = 4
        PW = TW // NP
        for k in range(NP):
            lo, hi = PW * k, PW * (k + 1) if k < NP - 1 else TW
            piece = bass.AP(n0d[:].tensor, lo, [[1, 128], [1, hi - lo]])
            eng = nc.sync if k % 2 == 0 else nc.scalar
            eng.dma_start(t_sb[:, lo:hi], piece)

        # ---- scalar ring: constants for the filter path; out stores later ----
        ident = const.tile([128, 128], f32)
        nc.scalar.dma_start(ident[:], ident_in)
        wic_sb = const.tile([DA, IRP], f32r)
        nc.scalar.dma_start(wic_sb[:], wic_full.bitcast(f32r))

        # s_b = FSCALE * amps_b / M  (M broadcast via ones-matmul)
        gmax_ps = pmisc.tile([128, 1], f32, tag="misc")
        nc.tensor.matmul(gmax_ps[:], lhsT=ones_row[:], rhs=gmax[:], start=True, stop=True)
        minv = const.tile([128, 1], f32)
        nc.vector.reciprocal(minv[:], gmax_ps[:])
        s_sb = const.tile([128, 1], f32)
        nc.vector.tensor_mul(s_sb[:], amps_sb[:], minv[:])
        nc.scalar.mul(s_sb[:], s_sb[:], FSCALE)

        xs = const.tile([128, DA], f32)
        nc.vector.tensor_scalar_mul(xs[:], x_aug[:], s_sb[:])
        xsT_ps = pmisc.tile([DA, 128], f32, tag="misc")
        nc.tensor.transpose(xsT_ps[:], xs[:], ident[:])
        xsT = const.tile([DA, 128], f32r)
        nc.vector.tensor_copy(xsT[:], xsT_ps[:])

        # filter rows fT[i, b] = sum_d W_ic[d, i] * xs[b, d]  (fp16, x FSCALE)
        fT = const.tile([128, IRP], f16)
        for c in range(8):
            fp = pft.tile([128, 128], f32, tag="fp")
            nc.tensor.matmul(
                fp[:],
                lhsT=wic_sb[:, 128 * c : 128 * (c + 1)],
                rhs=xsT[:],
                start=True,
                stop=True,
            )
            nc.vector.tensor_copy(fT[:, 128 * c : 128 * (c + 1)], fp[:])

        # ---- conv: out[b, t] = 2^-8 sum_i fT[i, b] * T[i%128, t + 128*(i//128)]
        for t in range(16):
            po = pconv.tile([128, 512], f32, tag="conv")
            for c in range(8):
                nc.tensor.matmul(
                    po[:],
                    lhsT=fT[:, 128 * c : 128 * (c + 1)],
                    rhs=t_sb[:, 128 * c + 512 * t : 128 * c + 512 * t + 512],
                    start=(c == 0),
                    stop=(c == 7),
                )
            ob = work.tile([128, 512], f32, tag="outbounce")
            nc.vector.tensor_scalar_mul(ob[:], po[:], 1.0 / FSCALE)
            nc.scalar.dma_start(out_noise[:, 512 * t : 512 * (t + 1)], ob[:])

    nc.compile()
    return nc


def _get_progs():
    if "nc1" not in _CACHE:
        _CACHE["nc1"] = _build_prog1()
        _CACHE["nc2"] = _build_prog2()
    return _CACHE["nc1"], _CACHE["nc2"]


def _get_crev16():
    """(CSCALE * C_rev) as fp16, chunk-relaid: [8192, 1024] -> [64, 128, 1024]."""
    if "crev16" not in _CACHE:
        crev = _build_crev()
        _CACHE["crev16"] = np.ascontiguousarray(
            (crev * CSCALE).astype(np.float16).reshape(64, 128, IRP)
        )
    return _CACHE["crev16"]


def _prep(inputs: dict) -> dict:
    p = {}
    if "ident" not in _CACHE:
        _CACHE["ident"] = np.ascontiguousarray(np.eye(128, dtype=np.float32))
    p["ident"] = _CACHE["ident"]
    p["vel"] = np.ascontiguousarray(np.asarray(inputs["vel_inputs"]), dtype=np.float32)
    p["K"] = np.ascontiguousarray(np.asarray(inputs["K"]), dtype=np.float32)
    p["eps"] = np.ascontiguousarray(np.asarray(inputs["eps"]), dtype=np.float32)
    w_coeff = np.asarray(inputs["W_coeff"], dtype=np.float32)
    b_coeff = np.asarray(inputs["b_coeff"], dtype=np.float32)
    w_aug_t = np.concatenate([w_coeff.T, b_coeff[:, None]], axis=1)  # [W, DA]
    # wsb[p, DA*c + d] = w_aug_t[128c + p, d], fp16
    p["wsb"] = np.ascontiguousarray(
        w_aug_t.reshape(64, 128, DA).transpose(1, 0, 2).reshape(128, 64 * DA)
    ).astype(np.float16)
    p["w_amps"] = np.ascontiguousarray(
        np.concatenate(
            [np.asarray(inputs["W_amps"], np.float32), np.asarray(inputs["b_amps"], np.float32)[:, None]],
            axis=0,
        )
    )
    p["w_mean"] = np.ascontiguousarray(
        np.concatenate(
            [np.asarray(inputs["W_mean"], np.float32), np.asarray(inputs["b_mean"], np.float32)[:, None]],
            axis=0,
        )
    )
    return p


def make_in_maps1(p: dict) -> list:
    crev16 = _get_crev16()  # [64, 128, IRP]
    maps = []
    for c in range(NCORES):
        csb = np.ascontiguousarray(
            crev16[:, :, BSH * c : BSH * (c + 1)]
            .transpose(1, 0, 2)
            .reshape(128, 64 * BSH)
        )
        sl = slice(BSH * c, BSH * (c + 1))
        maps.append(
            {
                "vel": np.ascontiguousarray(p["vel"][sl]),
                "kk": np.ascontiguousarray(p["K"][sl]),
                "eps_sh": np.ascontiguousarray(p["eps"][sl]),
                "wsb": p["wsb"],
                "csb": csb,
                "w_mean": p["w_mean"],
                "w_amps": p["w_amps"],
                "ident": p["ident"],
            }
        )
    return maps


def glue12(results1: list) -> tuple:
    """Host-side exchange: gather W_ic slices, max of shard maxima, mean row 0."""
    wic_full = np.ascontiguousarray(
        np.concatenate([r["wic_out"] for r in results1], axis=1)
    )  # [DA, 1024]
    m = np.max([r["lmax_out"][0, 0] for r in results1]).reshape(1, 1).astype(np.float32)
    mean = np.concatenate([r["mean_out"] for r in results1], axis=0)  # [B, 1]
    mean0 = np.ascontiguousarray(mean[0:1, 0:1])
    amps = [np.ascontiguousarray(r["amps_out"]) for r in results1]
    return wic_full, m, mean, mean0, amps


def make_in_maps2(
    p: dict, wic_full: np.ndarray, m: np.ndarray, mean0: np.ndarray, amps: list
) -> list:
    maps = []
    for c in range(NCORES):
        sl = slice(BSH * c, BSH * (c + 1))
        maps.append(
            {
                "vel": np.ascontiguousarray(p["vel"][sl]),
                "kk": np.ascontiguousarray(p["K"][sl]),
                "eps0": np.ascontiguousarray(p["eps"][0:1]),
                "amps_in": amps[c],
                "mean0_in": mean0,
                "m_in": m,
                "wic_full": wic_full,
                "ident": p["ident"],
            }
        )
    return maps


def kernel(**inputs):
    from concourse.bass_utils import run_bass_kernel_spmd

    nc1, nc2 = _get_progs()
    p = _prep(inputs)
    trace = os.environ.get("NOISE_KERNEL_TRACE", "0") == "1"
    core_ids = list(range(NCORES))

    res1 = run_bass_kernel_spmd(nc1, make_in_maps1(p), core_ids=core_ids, trace=trace)
    wic_full, m, mean, mean0, amps = glue12(res1.results)
    res2 = run_bass_kernel_spmd(
        nc2, make_in_maps2(p, wic_full, m, mean0, amps), core_ids=core_ids, trace=trace
    )
    _CACHE["last_result1"] = res1
    _CACHE["last_result2"] = res2
    out = np.concatenate([r["out_noise"] for r in res2.results], axis=0)
    return out, mean
